# revision 23
# baseline (speedup 1.0000x reference)
"""Trainium2 Bass kernel for nn_AttentionHead_80436147520097.

Single attention head, B=4 T=4096 D=1024 H=64:
    k,q,v = x@W+b;  S[t,s] = k_t . q_s / 8 (causal s<=t);  out = softmax_s(S) @ v

Sharding: 8 cores = 4 batches x 2 parity groups. Within a batch, the two
cores split the softmax (s) dimension by 128-row block parity: core p owns
s-blocks with (block % 2 == p). Parity divergence is pushed into host data
(x columns pair-swapped for p==1, per-parity diag masks), so all 8 cores
run ONE SPMD program. Each core emits partial unnormalized out [t, 65]
(col 64 = softmax denominator) over its s-half; host adds halves, divides,
adds bv, un-permutes.

Device math (bq is dropped: softmax is shift-invariant per t-row since the
final normalization divides by the same-shifted denominator; bv is added on
the host: out = num/den + bv; bk is folded into the fp8 k conversion):

- x is fed as fp8e4m3 (x8) plus an fp8 residual (xl8, own s-columns only).
- kq proj: one fp8 DoubleRow pass [W8k|W8q]^T x8 over full T -> PSUM
  [k;q][128, 512] per chunk -> DVE converts to fp8 (scale 16, +16*bk on k
  rows) giving k8 [64, T] and q8 rows; q8 own-columns are copied by an
  SBUF DMA into the zero-interleaved DR stationary q8z [64, task, 2, 128]
  (i=1 tile is zeros so DoubleRow contributes q8^T k8 only).
- v proj (error-sensitive, needs ~bf16 quality): 3-pass fp8 residual
  (x8@Wv8 + xl8@Wv8 + x8@Wvl8) with x-slices as the STATIONARY so the
  output lands v-natural [s=128, h] directly (no transposes).
- S^T[s-task, t-chunk] = DoubleRow(q8z[task], k8[chunk(+ghost)]) * (0.125/256)
- exp: Activation engine (Exp) for most (pair, chunk) tiles; a tunable
  fraction on DVE via Schraudolph fast-exp (x*c1+c2 -> int16 -> bf16 bits).
- diag masking: per chunk j, pair j is diagonal; e columns [0:256] and
  [768:1024] multiply a per-parity [128, 2, 256] mask ([tri|ones] for p=0,
  [tri|zeros] for p=1) -- uniform across tasks.
- PV flipped: out[t-block, 0:65] += e[s-task, t-block]^T @ [v_task|1]
  (65-wide moving operand), skipping dead blocks (m <= jd//2), PSUM group
  per t-block, DVE drains to SBUF outbuf, DMA out.
"""

import sys

import numpy as np

try:
    import ml_dtypes
except ImportError:  # pragma: no cover
    sys.path.insert(0, "/opt/trn_rl_repo")
    import ml_dtypes

B, T, D, H = 4, 4096, 1024, 64
NCORES = 8
NCHUNK = 8           # t-chunks of 512
NTASK = 16           # own s-tasks (128 rows each)
BF16 = ml_dtypes.bfloat16
F8 = ml_dtypes.float8_e4m3

KQ_SCALE = 16.0      # k8 = 16(k+bk), q8 = 16 q
S_SCALE = 0.125 / (KQ_SCALE * KQ_SCALE)
A_W = 512.0          # fp8 weight pre-scale (keeps W in e4m3 normal range)
G_XL = 16.0          # fp8 x-residual pre-scale
# fraction (in tenths) of exp tiles computed on DVE via Schraudolph
SCHRAUD_TENTHS = 3

_cache = {}


def _build_program():
    import concourse.bacc as bacc
    import concourse.mybir as mybir
    import concourse.tile as tile

    f32 = mybir.dt.float32
    bf16 = mybir.dt.bfloat16
    fp8 = mybir.dt.float8e4
    i16 = mybir.dt.int16
    DR = mybir.MatmulPerfMode.DoubleRow
    Exp = mybir.ActivationFunctionType.Exp

    LOG2E = 1.4426950408889634
    SCH_C1 = S_SCALE * LOG2E * 128.0
    SCH_C2 = 127.0 * 128.0 - 0.5 * 128.0 * 0.0579 + 0.5

    nc = bacc.Bacc("TRN2", target_bir_lowering=False, debug=False,
                   num_devices=NCORES)

    x8_d = nc.dram_tensor("x8", [D, T], fp8, kind="ExternalInput").ap()
    xl8_d = nc.dram_tensor("xl8", [D, T], fp8, kind="ExternalInput").ap()
    wkq_d = nc.dram_tensor("wkq", [D, 128], fp8, kind="ExternalInput").ap()
    wkqb_d = nc.dram_tensor("wkqb", [D, 128], fp8, kind="ExternalInput").ap()
    wkql_d = nc.dram_tensor("wkql", [D, 128], fp8, kind="ExternalInput").ap()
    wv8_d = nc.dram_tensor("wv8", [D, H], fp8, kind="ExternalInput").ap()
    wv8b_d = nc.dram_tensor("wv8b", [D, H], fp8, kind="ExternalInput").ap()
    wvl_d = nc.dram_tensor("wvl", [D, H], fp8, kind="ExternalInput").ap()
    bias_d = nc.dram_tensor("bias", [128, 1], f32, kind="ExternalInput").ap()
    mask_d = nc.dram_tensor("mask", [128, 512], bf16, kind="ExternalInput").ap()
    out_d = nc.dram_tensor("out", [T, 65], f32, kind="ExternalOutput").ap()

    with tile.TileContext(nc) as tc:
        with (
            tc.tile_pool(name="const", bufs=1) as const,
            tc.tile_pool(name="x8p", bufs=1) as x8p,
            tc.tile_pool(name="sb", bufs=1) as sb,
            tc.tile_pool(name="e", bufs=12) as e_p,
            tc.tile_pool(name="kq_ps", bufs=1, space="PSUM") as kq_psp,
            tc.tile_pool(name="v_ps", bufs=1, space="PSUM") as v_psp,
            tc.tile_pool(name="s_ps", bufs=2, space="PSUM") as s_psp,
            tc.tile_pool(name="pv_ps", bufs=1, space="PSUM") as pv_psp,
        ):
            # ---------------- constants ----------------
            wkq = const.tile([128, 8 * 128], fp8)   # [p, c, 128] = F(512[Wk|Wq])
            nc.sync.dma_start(
                wkq[:].rearrange("p (c m) -> p c m", c=8),
                wkq_d.rearrange("(c p) m -> p c m", p=128))
            wkqb = const.tile([128, 8 * 128], fp8)  # F(32[Wk|Wq]) for xl pass
            nc.sync.dma_start(
                wkqb[:].rearrange("p (c m) -> p c m", c=8),
                wkqb_d.rearrange("(c p) m -> p c m", p=128))
            wkql = const.tile([128, 8 * 128], fp8)  # F(512W - wkq) W-residual
            nc.sync.dma_start(
                wkql[:].rearrange("p (c m) -> p c m", c=8),
                wkql_d.rearrange("(c p) m -> p c m", p=128))
            x8 = x8p.tile([128, 8 * T], fp8)        # [p, c, t-device]
            x83 = x8[:].rearrange("p (c t) -> p c t", c=8)
            x8d3 = x8_d.rearrange("(c p) t -> p c t", p=128)
            xl8 = x8p.tile([128, 8 * T], fp8)       # 16*(x - x8), full T
            xl83 = xl8[:].rearrange("p (c t) -> p c t", c=8)
            xl8d3 = xl8_d.rearrange("(c p) t -> p c t", p=128)
            wv8 = const.tile([128, 8 * H], fp8)
            nc.sync.dma_start(
                wv8[:].rearrange("p (c m) -> p c m", c=8),
                wv8_d.rearrange("(c p) m -> p c m", p=128))
            wv8b = const.tile([128, 8 * H], fp8)
            nc.sync.dma_start(
                wv8b[:].rearrange("p (c m) -> p c m", c=8),
                wv8b_d.rearrange("(c p) m -> p c m", p=128))
            wvl = const.tile([128, 8 * H], fp8)
            nc.sync.dma_start(
                wvl[:].rearrange("p (c m) -> p c m", c=8),
                wvl_d.rearrange("(c p) m -> p c m", p=128))
            bias = const.tile([128, 1], f32)
            nc.sync.dma_start(bias[:], bias_d)
            masks = const.tile([128, 512], bf16)    # [p, 2, 256] per parity
            nc.sync.dma_start(masks[:], mask_d)

            # ---------------- persistent SBUF ----------------
            # kq8: rows 0:64 = k8 (full T + 512 ghost pad), rows 64:128 = q8
            kq8 = sb.tile([128, T + 512], fp8)
            q8z = sb.tile([64, NTASK * 256], fp8)   # [64, task, 2, 128], i=1 zeros
            # zero q8z (i=1 tiles MUST be 0) and all of kq8 (S-matmul ghost
            # reads must be finite before the next chunk's proj lands);
            # gpsimd = otherwise-idle engine
            nc.gpsimd.memset(q8z[:], 0.0)
            nc.gpsimd.memset(kq8[:], 0.0)
            v_nat = sb.tile([128, NTASK * 65], bf16)
            ones_col = v_nat[:].rearrange("p (n w) -> p n w", w=65)[:, :, 64:65]
            nc.vector.memset(ones_col, 1.0)
            outbuf = sb.tile([128, 32 * 65], f32)
            # prefetch exp table off the critical path
            scratch = const.tile([1, 8], f32)
            nc.vector.memset(scratch[:], 0.0)
            nc.scalar.activation(scratch[:], scratch[:], Exp)

            q8z4 = q8z[:].rearrange("p (n i m) -> p n i m", n=NTASK, i=2)
            wkq3 = wkq[:].rearrange("p (c m) -> p c m", c=8)
            wkqb3 = wkqb[:].rearrange("p (c m) -> p c m", c=8)
            wkql3 = wkql[:].rearrange("p (c m) -> p c m", c=8)
            wv83 = wv8[:].rearrange("p (c m) -> p c m", c=8)
            wv8b3 = wv8b[:].rearrange("p (c m) -> p c m", c=8)
            wvl3 = wvl[:].rearrange("p (c m) -> p c m", c=8)
            v3 = v_nat[:].rearrange("p (n w) -> p n w", w=65)

            def proj(j):
                t0 = 512 * j
                # --- kq projection: 3 fp8-DR residual passes over chunk j ---
                ps = kq_psp.tile([128, 512], f32, tag="kq")
                for u in range(4):
                    c2 = slice(2 * u, 2 * u + 2)
                    nc.tensor.matmul(
                        ps[:], wkq3[:, c2, :], x83[:, c2, t0:t0 + 512],
                        start=(u == 0), stop=False,
                        perf_mode=DR, skip_group_check=True)
                    nc.tensor.matmul(
                        ps[:], wkqb3[:, c2, :], xl83[:, c2, t0:t0 + 512],
                        start=False, stop=False,
                        perf_mode=DR, skip_group_check=True)
                    nc.tensor.matmul(
                        ps[:], wkql3[:, c2, :], x83[:, c2, t0:t0 + 512],
                        start=False, stop=(u == 3),
                        perf_mode=DR, skip_group_check=True)
                # psum holds A_W*[k;q]; convert to k8=16(k+bk), q8=16q
                nc.vector.tensor_scalar(
                    out=kq8[:, t0:t0 + 512], in0=ps[:],
                    scalar1=KQ_SCALE / A_W, scalar2=bias[:, 0:1],
                    op0=mybir.AluOpType.mult, op1=mybir.AluOpType.add)
                # q8z fill: own cols of chunk j (device blocks 0 and 2 of the
                # chunk) -> tasks 2j, 2j+1 slot i=0
                nc.sync.dma_start(
                    q8z4[:, 2 * j:2 * j + 2, 0:1, :],
                    kq8[64:128, t0:t0 + 512]
                    .rearrange("p (b m) -> p b m", b=4)[:, 0:3:2, :]
                    .rearrange("p b m -> p b () m"))
                # --- v projection for tasks 2j, 2j+1: 3-pass fp8 residual,
                # x-slice stationary -> v natural [s=128, h] ---
                for dm in range(2):
                    m = 2 * j + dm
                    vp = v_psp.tile([128, 512], f32, tag="v")
                    xo = x83[:, :, t0 + 256 * dm: t0 + 256 * dm + 128]
                    xlo = xl83[:, :, t0 + 256 * dm: t0 + 256 * dm + 128]
                    for u in range(4):
                        c2 = slice(2 * u, 2 * u + 2)
                        last = (u == 3)
                        nc.tensor.matmul(
                            vp[:, 0:64], xo[:, c2, :], wv83[:, c2, :],
                            start=(u == 0), stop=False,
                            perf_mode=DR, skip_group_check=True)
                        nc.tensor.matmul(
                            vp[:, 0:64], xlo[:, c2, :], wv8b3[:, c2, :],
                            start=False, stop=False,
                            perf_mode=DR, skip_group_check=True)
                        nc.tensor.matmul(
                            vp[:, 0:64], xo[:, c2, :], wvl3[:, c2, :],
                            start=False, stop=last,
                            perf_mode=DR, skip_group_check=True)
                    nc.vector.tensor_copy(out=v3[:, m, 0:64], in_=vp[:, 0:64])

            def attn(j):
                t0 = 512 * j
                # S + exp per pair P = (tasks 2P, 2P+1)
                es = []
                for P in range(j + 1):
                    ps = s_psp.tile([128, 1024], f32, tag="s")
                    e = e_p.tile([128, 1024], bf16, tag="e")
                    for h in range(2):
                        nc.tensor.matmul(
                            ps[:, 512 * h:512 * h + 512],
                            q8z4[:, 2 * P + h, :, :],
                            kq8[0:64, t0:t0 + 1024]
                            .rearrange("p (i n) -> p i n", i=2),
                            start=True, stop=True,
                            perf_mode=DR, skip_group_check=True)
                    if (P * 3 + j) % 10 < SCHRAUD_TENTHS and P != j:
                        nc.vector.tensor_scalar(
                            out=e[:].bitcast(i16), in0=ps[:],
                            scalar1=SCH_C1, scalar2=SCH_C2,
                            op0=mybir.AluOpType.mult,
                            op1=mybir.AluOpType.add)
                    else:
                        nc.scalar.activation(e[:], ps[:], Exp, scale=S_SCALE)
                    if P == j:  # diagonal pair: mask cols [0:256], [768:1024]
                        src = e[:].rearrange("p (a n) -> p a n", n=256)[:, 0:4:3, :]
                        nc.vector.tensor_mul(
                            src, src,
                            masks[:].rearrange("p (a n) -> p a n", n=256))
                    es.append(e)
                # PV per t-block pair
                for half in range(2):
                    po = pv_psp.tile([128, 1024], f32, tag="pv")
                    for dj in range(2):
                        jd = 4 * j + 2 * half + dj
                        for m in range(jd // 2 + 1):
                            e = es[m // 2]
                            col = 512 * (m % 2) + 128 * (jd % 4)
                            nc.tensor.matmul(
                                po[:, 512 * dj:512 * dj + 65],
                                e[:, col:col + 128],
                                v3[:, m, :],
                                start=(m == 0), stop=(m == jd // 2),
                                skip_group_check=True)
                    nc.vector.tensor_copy(
                        out=outbuf[:, 130 * (2 * j + half):
                                   130 * (2 * j + half) + 130],
                        in_=po[:].rearrange("p (i n) -> p i n", i=2)[:, :, 0:65])

            out_r = out_d.rearrange("(jd t) h -> t jd h", t=128)
            for j in range(NCHUNK):
                # stream x for this chunk
                nc.sync.dma_start(x83[:, :, 512 * j:512 * (j + 1)],
                                  x8d3[:, :, 512 * j:512 * (j + 1)])
                nc.sync.dma_start(xl83[:, :, 512 * j:512 * (j + 1)],
                                  xl8d3[:, :, 512 * j:512 * (j + 1)])
                proj(j)
                attn(j)
                if j % 2 == 1:  # flush 8 t-blocks
                    g = j // 2
                    nc.sync.dma_start(
                        out_r[:, 8 * g:8 * (g + 1), :],
                        outbuf[:, 520 * g:520 * (g + 1)]
                        .rearrange("p (jd h) -> p jd h", h=65))
    nc.compile()
    return nc


def _host_mask(p):
    """[128, 2, 256] bf16: [tri|ones] for p=0, [tri|zeros] for p=1."""
    s = np.arange(128)[:, None]
    c = np.arange(128)[None, :]
    tri = (s <= c).astype(np.float32)
    second = np.ones((128, 128), np.float32) if p == 0 else np.zeros((128, 128), np.float32)
    m = np.concatenate([tri, second], axis=1)  # [128, 256]
    return np.concatenate([m, m], axis=1).astype(BF16)  # [128, 512] = [2, 256]


def kernel(x, Wk, bk, Wq, bq, Wv, bv):
    from concourse.bass_utils import run_bass_kernel_spmd

    if "nc" not in _cache:
        _cache["nc"] = _build_program()
    nc = _cache["nc"]

    x = np.asarray(x, np.float32)
    wkq_f = A_W * np.concatenate([np.asarray(Wk), np.asarray(Wq)],
                                 axis=1).astype(np.float32)
    wkq = wkq_f.astype(F8)
    wkqb = (wkq_f / G_XL).astype(F8)
    wkql = (wkq_f - wkq.astype(np.float32)).astype(F8)
    wv_f = np.asarray(Wv, np.float32)
    wv8 = (A_W * wv_f).astype(F8)
    wv8b = (A_W / G_XL * wv_f).astype(F8)
    wvl = (A_W * wv_f - wv8.astype(np.float32)).astype(F8)
    bias = np.zeros((128, 1), np.float32)
    bias[0:64, 0] = KQ_SCALE * np.asarray(bk, np.float32)

    in_maps = []
    for core in range(NCORES):
        b, p = core // 2, core % 2
        xb = x[b]  # [T, D]
        if p == 1:  # pair-swap 128-row blocks within 256-row pairs
            xb = xb.reshape(T // 256, 2, 128, D)[:, ::-1].reshape(T, D)
        xT = np.ascontiguousarray(xb.T)        # [D, T-device]
        x8 = xT.astype(F8)
        xl8 = (G_XL * (xT - x8.astype(np.float32))).astype(F8)
        in_maps.append({
            "x8": x8,
            "xl8": xl8,
            "wkq": wkq,
            "wkqb": wkqb,
            "wkql": wkql,
            "wv8": wv8,
            "wv8b": wv8b,
            "wvl": wvl,
            "bias": bias,
            "mask": _host_mask(p),
        })

    res = run_bass_kernel_spmd(nc, in_maps, core_ids=list(range(NCORES)))
    results = res.results
    _cache["last_run"] = res

    bv_f = np.asarray(bv, np.float32)
    out = np.zeros((B, T, H), np.float32)
    for b in range(B):
        a0 = results[2 * b]["out"]      # [T-device, 65], device == orig for p=0
        a1 = results[2 * b + 1]["out"]  # [T-device, 65], pair-swapped
        a1 = a1.reshape(T // 256, 2, 128, 65)[:, ::-1].reshape(T, 65)
        tot = a0 + a1
        # v was carried at scale A_W on-device
        out[b] = tot[:, 0:64] / (A_W * tot[:, 64:65]) + bv_f
    return out


# revision 37
# speedup vs baseline: 1.0642x; 1.0642x over previous
"""Trainium2 Bass kernel for nn_AttentionHead_80436147520097.

Single attention head, B=4 T=4096 D=1024 H=64:
    k,q,v = x@W+b;  S[t,s] = k_t . q_s / 8 (causal s<=t);  out = softmax_s(S) @ v

Sharding: 8 cores = 4 batches x 2 parity groups. Within a batch, the two
cores split the softmax (s) dimension by 128-row block parity: core p owns
s-blocks with (block % 2 == p). Parity divergence is pushed into host data
(x columns pair-swapped for p==1, per-parity diag masks), so all 8 cores
run ONE SPMD program. Each core emits partial unnormalized out [t, 65]
(col 64 = softmax denominator) over its s-half; host adds halves, divides,
adds bv, un-permutes.

Device math (bq is dropped: softmax is shift-invariant per t-row since the
final normalization divides by the same-shifted denominator; bv is added on
the host: out = num/den + bv; bk is folded into the fp8 k conversion):

- x is fed as fp8e4m3 (x8) plus an fp8 residual (xl8, own s-columns only).
- kq proj: one fp8 DoubleRow pass [W8k|W8q]^T x8 over full T -> PSUM
  [k;q][128, 512] per chunk -> DVE converts to fp8 (scale 16, +16*bk on k
  rows) giving k8 [64, T] and q8 rows; q8 own-columns are copied by an
  SBUF DMA into the zero-interleaved DR stationary q8z [64, task, 2, 128]
  (i=1 tile is zeros so DoubleRow contributes q8^T k8 only).
- v proj (error-sensitive, needs ~bf16 quality): 3-pass fp8 residual
  (x8@Wv8 + xl8@Wv8 + x8@Wvl8) with x-slices as the STATIONARY so the
  output lands v-natural [s=128, h] directly (no transposes).
- S^T[s-task, t-chunk] = DoubleRow(q8z[task], k8[chunk(+ghost)]) * (0.125/256)
- exp: Activation engine (Exp) for most (pair, chunk) tiles; a tunable
  fraction on DVE via Schraudolph fast-exp (x*c1+c2 -> int16 -> bf16 bits).
- diag masking: per chunk j, pair j is diagonal; e columns [0:256] and
  [768:1024] multiply a per-parity [128, 2, 256] mask ([tri|ones] for p=0,
  [tri|zeros] for p=1) -- uniform across tasks.
- PV flipped: out[t-block, 0:65] += e[s-task, t-block]^T @ [v_task|1]
  (65-wide moving operand), skipping dead blocks (m <= jd//2), PSUM group
  per t-block, DVE drains to SBUF outbuf, DMA out.
"""

import sys

import numpy as np

try:
    import ml_dtypes
except ImportError:  # pragma: no cover
    sys.path.insert(0, "/opt/trn_rl_repo")
    import ml_dtypes

B, T, D, H = 4, 4096, 1024, 64
NCORES = 8
NCHUNK = 8           # t-chunks of 512
NTASK = 16           # own s-tasks (128 rows each)
BF16 = ml_dtypes.bfloat16
F8 = ml_dtypes.float8_e4m3

KQ_SCALE = 16.0      # k8 = 16(k+bk), q8 = 16 q
S_SCALE = 0.125 / (KQ_SCALE * KQ_SCALE)
A_W = 512.0          # fp8 weight pre-scale (keeps W in e4m3 normal range)
G_XL = 16.0          # fp8 x-residual pre-scale
# fraction (in tenths) of exp tiles computed on DVE via Schraudolph
SCHRAUD_TENTHS = 3

_cache = {}


def _build_program():
    import concourse.bacc as bacc
    import concourse.mybir as mybir
    import concourse.tile as tile

    f32 = mybir.dt.float32
    bf16 = mybir.dt.bfloat16
    fp8 = mybir.dt.float8e4
    i16 = mybir.dt.int16
    DR = mybir.MatmulPerfMode.DoubleRow
    Exp = mybir.ActivationFunctionType.Exp

    LOG2E = 1.4426950408889634
    SCH_C1 = S_SCALE * LOG2E * 128.0
    SCH_C2 = 127.0 * 128.0 - 0.5 * 128.0 * 0.0579 + 0.5

    nc = bacc.Bacc("TRN2", target_bir_lowering=False, debug=False,
                   num_devices=NCORES)

    x8_d = nc.dram_tensor("x8", [D, T], fp8, kind="ExternalInput").ap()
    xl8_d = nc.dram_tensor("xl8", [D, T], fp8, kind="ExternalInput").ap()
    wpack_d = nc.dram_tensor("wpack", [128, 4608], fp8, kind="ExternalInput").ap()
    bias_d = nc.dram_tensor("bias", [128, 1], f32, kind="ExternalInput").ap()
    mask_d = nc.dram_tensor("mask", [128, 512], bf16, kind="ExternalInput").ap()
    out_d = nc.dram_tensor("out", [T, 65], f32, kind="ExternalOutput").ap()

    with tile.TileContext(nc) as tc:
        with (
            tc.tile_pool(name="const", bufs=1) as const,
            tc.tile_pool(name="x8p", bufs=1) as x8p,
            tc.tile_pool(name="sb", bufs=1) as sb,
            tc.tile_pool(name="e", bufs=20) as e_p,
            tc.tile_pool(name="proj_ps", bufs=2, space="PSUM") as proj_psp,
            tc.tile_pool(name="s_ps", bufs=2, space="PSUM") as s_psp,
            tc.tile_pool(name="pv_ps", bufs=1, space="PSUM") as pv_psp,
        ):
            # ---------------- constants ----------------
            # one pre-shuffled weight pack: [wkq|wkqb|wkql (1024 each) |
            # wv8|wv8b|wvl (512 each)], all [p, c, m] c-major
            wpack = const.tile([128, 4608], fp8)
            nc.sync.dma_start(wpack[:], wpack_d)
            x8 = x8p.tile([128, 8 * T], fp8)        # [p, c, t-device]
            x83 = x8[:].rearrange("p (c t) -> p c t", c=8)
            x8d3 = x8_d.rearrange("(c p) t -> p c t", p=128)
            xl8 = x8p.tile([128, 8 * T], fp8)       # 16*(x - x8), full T
            xl83 = xl8[:].rearrange("p (c t) -> p c t", c=8)
            xl8d3 = xl8_d.rearrange("(c p) t -> p c t", p=128)
            bias = const.tile([128, 1], f32)
            masks = const.tile([128, 512], bf16)    # [p, 2, 256] per parity

            # ---------------- persistent SBUF ----------------
            # kq8 [128, chunk, 1024]: cols [1024j, 1024j+512) = chunk j
            # (rows 0:64 = k8, 64:128 = q8); cols [+512, +1024) = that
            # chunk's dedicated DR-ghost pad (must be finite, never real)
            kq8 = sb.tile([128, NCHUNK * 1024], fp8)
            kq83 = kq8[:].rearrange("p (j n) -> p j n", j=NCHUNK)
            q8z = sb.tile([64, NTASK * 256], fp8)   # [64, task, 2, 128], i=1 zeros
            # i=1 slots of q8z MUST be 0; pads must be finite. Small separate
            # memsets on gpsimd (idle engine) so nothing stalls on one big op.
            nc.gpsimd.memset(
                q8z[:].rearrange("p (n i m) -> p n i m", n=NTASK, i=2)[:, :, 1:2, :],
                0.0)
            for j in range(NCHUNK):
                nc.gpsimd.memset(kq83[:, j, 512:1024], 0.0)
            v_nat = sb.tile([128, NTASK * 65], bf16)
            ones_col = v_nat[:].rearrange("p (n w) -> p n w", w=65)[:, :, 64:65]
            nc.vector.memset(ones_col, 1.0)
            outbuf = sb.tile([128, 32 * 65], f32)
            # prefetch exp table off the critical path
            scratch = const.tile([1, 8], f32)
            nc.vector.memset(scratch[:], 0.0)
            nc.scalar.activation(scratch[:], scratch[:], Exp)

            q8z4 = q8z[:].rearrange("p (n i m) -> p n i m", n=NTASK, i=2)
            wkq3 = wpack[:, 0:1024].rearrange("p (c m) -> p c m", c=8)
            wkqb3 = wpack[:, 1024:2048].rearrange("p (c m) -> p c m", c=8)
            wkql3 = wpack[:, 2048:3072].rearrange("p (c m) -> p c m", c=8)
            wv83 = wpack[:, 3072:3584].rearrange("p (c m) -> p c m", c=8)
            wv8b3 = wpack[:, 3584:4096].rearrange("p (c m) -> p c m", c=8)
            wvl3 = wpack[:, 4096:4608].rearrange("p (c m) -> p c m", c=8)
            v3 = v_nat[:].rearrange("p (n w) -> p n w", w=65)

            def proj(j):
                t0 = 512 * j
                # --- kq projection: 3 fp8-DR residual passes over chunk j ---
                ps = proj_psp.tile([128, 512], f32, tag="proj")
                for u in range(4):
                    c2 = slice(2 * u, 2 * u + 2)
                    nc.tensor.matmul(
                        ps[:], wkq3[:, c2, :], x83[:, c2, t0:t0 + 512],
                        start=(u == 0), stop=False,
                        perf_mode=DR, skip_group_check=True)
                    nc.tensor.matmul(
                        ps[:], wkqb3[:, c2, :], xl83[:, c2, t0:t0 + 512],
                        start=False, stop=False,
                        perf_mode=DR, skip_group_check=True)
                    nc.tensor.matmul(
                        ps[:], wkql3[:, c2, :], x83[:, c2, t0:t0 + 512],
                        start=False, stop=(u == 3),
                        perf_mode=DR, skip_group_check=True)
                # psum holds A_W*[k;q]; convert to k8=16(k+bk), q8=16q
                nc.vector.tensor_scalar(
                    out=kq83[:, j, 0:512], in0=ps[:],
                    scalar1=KQ_SCALE / A_W, scalar2=bias[:, 0:1],
                    op0=mybir.AluOpType.mult, op1=mybir.AluOpType.add)
                # q8z fill: own cols of chunk j (device blocks 0 and 2 of the
                # chunk) -> tasks 2j, 2j+1 slot i=0.  ACT queue: its wait
                # (kq convert) must not block SP's x prefetches.
                nc.sync.dma_start(
                    q8z4[:, 2 * j:2 * j + 2, 0:1, :],
                    kq83[:, j, 0:512][64:128, :]
                    .rearrange("p (b m) -> p b m", b=4)[:, 0:3:2, :]
                    .rearrange("p b m -> p b () m"))
                # --- v projection for tasks 2j, 2j+1: 3-pass fp8 residual,
                # x-slice stationary -> v natural [s=128, h] ---
                for dm in range(2):
                    m = 2 * j + dm
                    vp = proj_psp.tile([128, 512], f32, tag="proj")
                    xo = x83[:, :, t0 + 256 * dm: t0 + 256 * dm + 128]
                    xlo = xl83[:, :, t0 + 256 * dm: t0 + 256 * dm + 128]
                    for u in range(4):
                        c2 = slice(2 * u, 2 * u + 2)
                        last = (u == 3)
                        nc.tensor.matmul(
                            vp[:, 0:64], xo[:, c2, :], wv83[:, c2, :],
                            start=(u == 0), stop=False,
                            perf_mode=DR, skip_group_check=True)
                        nc.tensor.matmul(
                            vp[:, 0:64], xlo[:, c2, :], wv8b3[:, c2, :],
                            start=False, stop=False,
                            perf_mode=DR, skip_group_check=True)
                        nc.tensor.matmul(
                            vp[:, 0:64], xo[:, c2, :], wvl3[:, c2, :],
                            start=False, stop=last,
                            perf_mode=DR, skip_group_check=True)
                    nc.vector.tensor_copy(out=v3[:, m, 0:64], in_=vp[:, 0:64])

            e_tiles = {}

            def attn_s(j):
                # S + exp per pair P = (tasks 2P, 2P+1)
                es = e_tiles[j] = []
                for P in range(j + 1):
                    ps = s_psp.tile([128, 1024], f32, tag="s")
                    e = e_p.tile([128, 1024], bf16, tag="e")
                    for h in range(2):
                        nc.tensor.matmul(
                            ps[:, 512 * h:512 * h + 512],
                            q8z4[:, 2 * P + h, :, :],
                            kq83[:, j, :][0:64, :]
                            .rearrange("p (i n) -> p i n", i=2),
                            start=True, stop=True,
                            perf_mode=DR, skip_group_check=True)
                    if (P * 3 + j) % 10 < SCHRAUD_TENTHS and P != j:
                        nc.vector.tensor_scalar(
                            out=e[:].bitcast(i16), in0=ps[:],
                            scalar1=SCH_C1, scalar2=SCH_C2,
                            op0=mybir.AluOpType.mult,
                            op1=mybir.AluOpType.add)
                    else:
                        nc.scalar.activation(e[:], ps[:], Exp, scale=S_SCALE)
                    if P == j:  # diagonal pair: mask cols [0:256], [768:1024]
                        src = e[:].rearrange("p (a n) -> p a n", n=256)[:, 0:4:3, :]
                        nc.gpsimd.tensor_mul(
                            src, src,
                            masks[:].rearrange("p (a n) -> p a n", n=256))
                    es.append(e)

            def attn_pv(j):
                es = e_tiles.pop(j)
                # PV per t-block pair
                for half in range(2):
                    po = pv_psp.tile([128, 1024], f32, tag="pv")
                    for dj in range(2):
                        jd = 4 * j + 2 * half + dj
                        for m in range(jd // 2 + 1):
                            e = es[m // 2]
                            col = 512 * (m % 2) + 128 * (jd % 4)
                            nc.tensor.matmul(
                                po[:, 512 * dj:512 * dj + 65],
                                e[:, col:col + 128],
                                v3[:, m, :],
                                start=(m == 0), stop=(m == jd // 2),
                                skip_group_check=True)
                    nc.vector.tensor_copy(
                        out=outbuf[:, 130 * (2 * j + half):
                                   130 * (2 * j + half) + 130],
                        in_=po[:].rearrange("p (i n) -> p i n", i=2)[:, :, 0:65])

            out_r = out_d.rearrange("(jd t) h -> t jd h", t=128)
            def load_x(j):
                nc.sync.dma_start(x83[:, :, 512 * j:512 * (j + 1)],
                                  x8d3[:, :, 512 * j:512 * (j + 1)])
                nc.sync.dma_start(xl83[:, :, 512 * j:512 * (j + 1)],
                                  xl8d3[:, :, 512 * j:512 * (j + 1)])

            def flush_out(j):
                if j % 2 == 1:  # flush 8 t-blocks
                    g = j // 2
                    nc.sync.dma_start(
                        out_r[:, 8 * g:8 * (g + 1), :],
                        outbuf[:, 520 * g:520 * (g + 1)]
                        .rearrange("p (jd h) -> p jd h", h=65))

            load_x(0)
            nc.sync.dma_start(bias[:], bias_d)
            nc.sync.dma_start(masks[:], mask_d)
            load_x(1)
            # software-pipelined depth 2: proj leads attn_s by one chunk
            # (so the kq convert isn't queued on DVE behind exp/drain work)
            # and attn_pv trails attn_s by one chunk (so the in-order PE
            # queue never waits on exp before the next chunk's proj/S).
            proj(0)
            for j in range(NCHUNK):
                if j + 2 < NCHUNK:
                    load_x(j + 2)
                if j + 1 < NCHUNK:
                    proj(j + 1)
                attn_s(j)
                if j > 0:
                    attn_pv(j - 1)
                    flush_out(j - 1)
            attn_pv(NCHUNK - 1)
            flush_out(NCHUNK - 1)
    nc.compile()
    return nc


def _host_mask(p):
    """[128, 2, 256] bf16: [tri|ones] for p=0, [tri|zeros] for p=1."""
    s = np.arange(128)[:, None]
    c = np.arange(128)[None, :]
    tri = (s <= c).astype(np.float32)
    second = np.ones((128, 128), np.float32) if p == 0 else np.zeros((128, 128), np.float32)
    m = np.concatenate([tri, second], axis=1)  # [128, 256]
    return np.concatenate([m, m], axis=1).astype(BF16)  # [128, 512] = [2, 256]


def kernel(x, Wk, bk, Wq, bq, Wv, bv):
    from concourse.bass_utils import run_bass_kernel_spmd

    if "nc" not in _cache:
        _cache["nc"] = _build_program()
    nc = _cache["nc"]

    x = np.asarray(x, np.float32)
    wkq_f = A_W * np.concatenate([np.asarray(Wk), np.asarray(Wq)],
                                 axis=1).astype(np.float32)
    wkq = wkq_f.astype(F8)
    wkqb = (wkq_f / G_XL).astype(F8)
    wkql = (wkq_f - wkq.astype(np.float32)).astype(F8)
    wv_f = np.asarray(Wv, np.float32)
    wv8 = (A_W * wv_f).astype(F8)
    wv8b = (A_W / G_XL * wv_f).astype(F8)
    wvl = (A_W * wv_f - wv8.astype(np.float32)).astype(F8)
    shuf = lambda w: np.ascontiguousarray(
        w.reshape(8, 128, -1).transpose(1, 0, 2).reshape(128, -1))
    wpack = np.concatenate(
        [shuf(w) for w in (wkq, wkqb, wkql, wv8, wv8b, wvl)], axis=1)
    bias = np.zeros((128, 1), np.float32)
    bias[0:64, 0] = KQ_SCALE * np.asarray(bk, np.float32)

    in_maps = []
    for core in range(NCORES):
        b, p = core // 2, core % 2
        xb = x[b]  # [T, D]
        if p == 1:  # pair-swap 128-row blocks within 256-row pairs
            xb = xb.reshape(T // 256, 2, 128, D)[:, ::-1].reshape(T, D)
        xT = np.ascontiguousarray(xb.T)        # [D, T-device]
        x8 = xT.astype(F8)
        xl8 = (G_XL * (xT - x8.astype(np.float32))).astype(F8)
        in_maps.append({
            "x8": x8,
            "xl8": xl8,
            "wpack": wpack,
            "bias": bias,
            "mask": _host_mask(p),
        })

    res = run_bass_kernel_spmd(nc, in_maps, core_ids=list(range(NCORES)))
    results = res.results
    _cache["last_run"] = res

    bv_f = np.asarray(bv, np.float32)
    out = np.zeros((B, T, H), np.float32)
    for b in range(B):
        a0 = results[2 * b]["out"]      # [T-device, 65], device == orig for p=0
        a1 = results[2 * b + 1]["out"]  # [T-device, 65], pair-swapped
        a1 = a1.reshape(T // 256, 2, 128, 65)[:, ::-1].reshape(T, 65)
        tot = a0 + a1
        # v was carried at scale A_W on-device
        out[b] = tot[:, 0:64] / (A_W * tot[:, 64:65]) + bv_f
    return out


# revision 52
# speedup vs baseline: 1.1650x; 1.0947x over previous
"""Trainium2 Bass kernel for nn_AttentionHead_80436147520097.

Single attention head, B=4 T=4096 D=1024 H=64:
    k,q,v = x@W+b;  S[t,s] = k_t . q_s / 8 (causal s<=t);  out = softmax_s(S) @ v

Sharding: 8 cores = 4 batches x 2 parity groups. Within a batch, the two
cores split the softmax (s) dimension by 128-row block parity: core p owns
s-blocks with (block % 2 == p). Parity divergence is pushed into host data
(x columns pair-swapped for p==1, per-parity diag masks), so all 8 cores
run ONE SPMD program. Each core emits partial unnormalized out [t, 65]
(col 64 = softmax denominator) over its s-half; host adds halves, divides,
adds bv, un-permutes.

Device math (bq is dropped: softmax is shift-invariant per t-row since the
final normalization divides by the same-shifted denominator; bv is added on
the host: out = num/den + bv; bk is folded into the fp8 k conversion):

- x is fed as fp8e4m3 (x8) plus an fp8 residual (xl8, own s-columns only).
- kq proj: one fp8 DoubleRow pass [W8k|W8q]^T x8 over full T -> PSUM
  [k;q][128, 512] per chunk -> DVE converts to fp8 (scale 16, +16*bk on k
  rows) giving k8 [64, T] and q8 rows; q8 own-columns are copied by an
  SBUF DMA into the zero-interleaved DR stationary q8z [64, task, 2, 128]
  (i=1 tile is zeros so DoubleRow contributes q8^T k8 only).
- v proj (error-sensitive, needs ~bf16 quality): 3-pass fp8 residual
  (x8@Wv8 + xl8@Wv8 + x8@Wvl8) with x-slices as the STATIONARY so the
  output lands v-natural [s=128, h] directly (no transposes).
- S^T[s-task, t-chunk] = DoubleRow(q8z[task], k8[chunk(+ghost)]) * (0.125/256)
- exp: Activation engine (Exp) for most (pair, chunk) tiles; a tunable
  fraction on DVE via Schraudolph fast-exp (x*c1+c2 -> int16 -> bf16 bits).
- diag masking: per chunk j, pair j is diagonal; e columns [0:256] and
  [768:1024] multiply a per-parity [128, 2, 256] mask ([tri|ones] for p=0,
  [tri|zeros] for p=1) -- uniform across tasks.
- PV flipped: out[t-block, 0:65] += e[s-task, t-block]^T @ [v_task|1]
  (65-wide moving operand), skipping dead blocks (m <= jd//2), PSUM group
  per t-block, DVE drains to SBUF outbuf, DMA out.
"""

import sys

import numpy as np

try:
    import ml_dtypes
except ImportError:  # pragma: no cover
    sys.path.insert(0, "/opt/trn_rl_repo")
    import ml_dtypes

B, T, D, H = 4, 4096, 1024, 64
NCORES = 8
NCHUNK = 8           # t-chunks of 512
NTASK = 16           # own s-tasks (128 rows each)
BF16 = ml_dtypes.bfloat16
F8 = ml_dtypes.float8_e4m3

KQ_SCALE = 16.0      # k8 = 16(k+bk), q8 = 16 q
S_SCALE = 0.125 / (KQ_SCALE * KQ_SCALE)
A_W = 512.0          # fp8 weight pre-scale (keeps W in e4m3 normal range)
G_XL = 16.0          # fp8 x-residual pre-scale
# fraction (in tenths) of exp tiles computed on DVE via Schraudolph
SCHRAUD_TENTHS = 3

_cache = {}


def _build_program():
    import concourse.bacc as bacc
    import concourse.mybir as mybir
    import concourse.tile as tile

    f32 = mybir.dt.float32
    bf16 = mybir.dt.bfloat16
    fp8 = mybir.dt.float8e4
    i16 = mybir.dt.int16
    DR = mybir.MatmulPerfMode.DoubleRow
    Exp = mybir.ActivationFunctionType.Exp

    LOG2E = 1.4426950408889634
    SCH_C1 = S_SCALE * LOG2E * 128.0
    SCH_C2 = 127.0 * 128.0 - 0.5 * 128.0 * 0.0579 + 0.5

    nc = bacc.Bacc("TRN2", target_bir_lowering=False, debug=False,
                   num_devices=NCORES)

    x8_d = nc.dram_tensor("x8", [D, T], fp8, kind="ExternalInput").ap()
    xl8_d = nc.dram_tensor("xl8", [D, T], fp8, kind="ExternalInput").ap()
    wpack_d = nc.dram_tensor("wpack", [128, 4608], fp8, kind="ExternalInput").ap()
    bias_d = nc.dram_tensor("bias", [128, 1], f32, kind="ExternalInput").ap()
    mask_d = nc.dram_tensor("mask", [128, 512], bf16, kind="ExternalInput").ap()
    out_d = nc.dram_tensor("out", [T, 65], f32, kind="ExternalOutput").ap()

    with tile.TileContext(nc) as tc:
        with (
            tc.tile_pool(name="const", bufs=1) as const,
            tc.tile_pool(name="x8p", bufs=1) as x8p,
            tc.tile_pool(name="sb", bufs=1) as sb,
            tc.tile_pool(name="e", bufs=20) as e_p,
            tc.tile_pool(name="proj_ps", bufs=1, space="PSUM") as proj_psp,
            tc.tile_pool(name="s_ps", bufs=2, space="PSUM") as s_psp,
            tc.tile_pool(name="s_ps2", bufs=1, space="PSUM") as s_psp2,
            tc.tile_pool(name="pv_ps", bufs=1, space="PSUM") as pv_psp,
        ):
            # ---------------- constants ----------------
            # one pre-shuffled weight pack: [wkq|wkqb|wkql (1024 each) |
            # wv8|wv8b|wvl (512 each)], all [p, c, m] c-major
            wpack = const.tile([128, 4608], fp8)
            nc.sync.dma_start(wpack[:], wpack_d)
            x8 = x8p.tile([128, 8 * T], fp8)        # [p, c, t-device]
            x83 = x8[:].rearrange("p (c t) -> p c t", c=8)
            x8d3 = x8_d.rearrange("(c p) t -> p c t", p=128)
            xl8 = x8p.tile([128, 8 * T], fp8)       # 16*(x - x8), full T
            xl83 = xl8[:].rearrange("p (c t) -> p c t", c=8)
            xl8d3 = xl8_d.rearrange("(c p) t -> p c t", p=128)
            bias = const.tile([128, 1], f32)
            masks = const.tile([128, 512], bf16)    # [p, 2, 256] per parity

            # ---------------- persistent SBUF ----------------
            # kq8 [128, chunk, 1024]: cols [1024j, 1024j+512) = chunk j
            # (rows 0:64 = k8, 64:128 = q8); cols [+512, +1024) = that
            # chunk's dedicated DR-ghost pad (must be finite, never real)
            kq8 = sb.tile([128, NCHUNK * 1024], fp8)
            kq83 = kq8[:].rearrange("p (j n) -> p j n", j=NCHUNK)
            q8z = sb.tile([64, NTASK * 256], fp8)   # [64, task, 2, 128], i=1 zeros
            # i=1 slots of q8z MUST be 0; pads must be finite. Small separate
            # memsets on gpsimd (idle engine) so nothing stalls on one big op.
            nc.gpsimd.memset(
                q8z[:].rearrange("p (n i m) -> p n i m", n=NTASK, i=2)[:, :, 1:2, :],
                0.0)
            for j in range(NCHUNK):
                nc.gpsimd.memset(kq83[:, j, 512:1024], 0.0)
            v_nat = sb.tile([128, NTASK * 65], bf16)
            ones_col = v_nat[:].rearrange("p (n w) -> p n w", w=65)[:, :, 64:65]
            nc.vector.memset(ones_col, 1.0)
            outbuf = sb.tile([128, 32 * 65], f32)
            # prefetch exp table off the critical path
            scratch = const.tile([1, 8], f32)
            nc.vector.memset(scratch[:], 0.0)
            nc.scalar.activation(scratch[:], scratch[:], Exp)

            q8z4 = q8z[:].rearrange("p (n i m) -> p n i m", n=NTASK, i=2)
            wkq3 = wpack[:, 0:1024].rearrange("p (c m) -> p c m", c=8)
            wkqb3 = wpack[:, 1024:2048].rearrange("p (c m) -> p c m", c=8)
            wkql3 = wpack[:, 2048:3072].rearrange("p (c m) -> p c m", c=8)
            wv83 = wpack[:, 3072:3584].rearrange("p (c m) -> p c m", c=8)
            wv8b3 = wpack[:, 3584:4096].rearrange("p (c m) -> p c m", c=8)
            wvl3 = wpack[:, 4096:4608].rearrange("p (c m) -> p c m", c=8)
            v3 = v_nat[:].rearrange("p (n w) -> p n w", w=65)

            def proj(j):
                t0 = 512 * j
                # --- kq projection: 3 fp8-DR residual passes over chunk j ---
                ps = proj_psp.tile([128, 512], f32, tag="proj")
                for pi, (w3, xs) in enumerate(
                        ((wkq3, x83), (wkql3, x83), (wkqb3, xl83))):
                    for u in range(4):
                        c2 = slice(2 * u, 2 * u + 2)
                        nc.tensor.matmul(
                            ps[:], w3[:, c2, :], xs[:, c2, t0:t0 + 512],
                            start=(pi == 0 and u == 0),
                            stop=(pi == 2 and u == 3),
                            perf_mode=DR, skip_group_check=True)
                # psum holds A_W*[k;q]; convert to k8=16(k+bk), q8=16q
                nc.vector.tensor_scalar(
                    out=kq83[:, j, 0:512], in0=ps[:],
                    scalar1=KQ_SCALE / A_W, scalar2=bias[:, 0:1],
                    op0=mybir.AluOpType.mult, op1=mybir.AluOpType.add)
                # q8z fill: own cols of chunk j (device blocks 0 and 2 of the
                # chunk) -> tasks 2j, 2j+1 slot i=0.  ACT queue: its wait
                # (kq convert) must not block SP's x prefetches.
                nc.sync.dma_start(
                    q8z4[:, 2 * j:2 * j + 2, 0:1, :],
                    kq83[:, j, 0:512][64:128, :]
                    .rearrange("p (b m) -> p b m", b=4)[:, 0:3:2, :]
                    .rearrange("p b m -> p b () m"))
                # --- v projection for tasks 2j, 2j+1: 3-pass fp8 residual,
                # x-slice stationary -> v natural [s=128, h] ---
                for dm in range(2):
                    m = 2 * j + dm
                    vp = proj_psp.tile([128, 512], f32, tag="proj")
                    xo = x83[:, :, t0 + 256 * dm: t0 + 256 * dm + 128]
                    xlo = xl83[:, :, t0 + 256 * dm: t0 + 256 * dm + 128]
                    for pi, (xs, w3) in enumerate(
                            ((xo, wv83), (xo, wvl3), (xlo, wv8b3))):
                        for u in range(4):
                            c2 = slice(2 * u, 2 * u + 2)
                            nc.tensor.matmul(
                                vp[:, 0:64], xs[:, c2, :], w3[:, c2, :],
                                start=(pi == 0 and u == 0),
                                stop=(pi == 2 and u == 3),
                                perf_mode=DR, skip_group_check=True)
                    nc.vector.tensor_copy(out=v3[:, m, 0:64], in_=vp[:, 0:64])

            e_tiles = {}

            def attn_s(j):
                # S + exp per pair P = (tasks 2P, 2P+1)
                es = e_tiles[j] = []
                for P in range(j + 1):
                    pool = s_psp2 if P % 3 == 2 else s_psp
                    ps = pool.tile([128, 1024], f32, tag="s")
                    e = e_p.tile([128, 1024], bf16, tag="e")
                    for h in range(2):
                        nc.tensor.matmul(
                            ps[:, 512 * h:512 * h + 512],
                            q8z4[:, 2 * P + h, :, :],
                            kq83[:, j, :][0:64, :]
                            .rearrange("p (i n) -> p i n", i=2),
                            start=True, stop=True,
                            perf_mode=DR, skip_group_check=True)
                    if (P * 3 + j) % 10 < SCHRAUD_TENTHS and P != j:
                        nc.vector.tensor_scalar(
                            out=e[:].bitcast(i16), in0=ps[:],
                            scalar1=SCH_C1, scalar2=SCH_C2,
                            op0=mybir.AluOpType.mult,
                            op1=mybir.AluOpType.add)
                    else:
                        nc.scalar.activation(e[:], ps[:], Exp, scale=S_SCALE)
                    if P == j:  # diagonal pair: mask cols [0:256], [768:1024]
                        src = e[:].rearrange("p (a n) -> p a n", n=256)[:, 0:4:3, :]
                        nc.gpsimd.tensor_mul(
                            src, src,
                            masks[:].rearrange("p (a n) -> p a n", n=256))
                    es.append(e)

            def pv_group(j, dj, tail=False):
                es = e_tiles[j]
                jd = 4 * j + dj
                if tail:  # S rings are idle after the last exp
                    pool = s_psp2 if dj % 3 == 2 else s_psp
                    po = pool.tile([128, 1024], f32, tag="s")
                else:
                    po = pv_psp.tile([128, 512], f32, tag="pv")
                for m in range(jd // 2 + 1):
                    e = es[m // 2]
                    col = 512 * (m % 2) + 128 * (jd % 4)
                    nc.tensor.matmul(
                        po[:, 0:65],
                        e[:, col:col + 128],
                        v3[:, m, :],
                        start=(m == 0), stop=(m == jd // 2),
                        skip_group_check=True)
                nc.vector.tensor_copy(
                    out=outbuf[:, 65 * jd:65 * jd + 65],
                    in_=po[:, 0:65])

            def attn_pv(j, tail=False):
                for dj in range(4):
                    pv_group(j, dj, tail=tail)
                e_tiles.pop(j)

            out_r = out_d.rearrange("(jd t) h -> t jd h", t=128)
            def load_x(j):
                # split by c-halves so the first proj matmuls can start
                # after a quarter of the chunk's bytes have landed
                for ch in range(2):
                    cs = slice(4 * ch, 4 * ch + 4)
                    nc.sync.dma_start(x83[:, cs, 512 * j:512 * (j + 1)],
                                      x8d3[:, cs, 512 * j:512 * (j + 1)])
                    nc.sync.dma_start(xl83[:, cs, 512 * j:512 * (j + 1)],
                                      xl8d3[:, cs, 512 * j:512 * (j + 1)])

            def flush_out(j):
                if j % 2 == 1:  # flush 8 t-blocks
                    g = j // 2
                    nc.sync.dma_start(
                        out_r[:, 8 * g:8 * (g + 1), :],
                        outbuf[:, 520 * g:520 * (g + 1)]
                        .rearrange("p (jd h) -> p jd h", h=65))

            load_x(0)
            nc.sync.dma_start(bias[:], bias_d)
            nc.sync.dma_start(masks[:], mask_d)
            load_x(1)
            # software-pipelined: attn_s(j) issues right after proj(j) so
            # the first S/exp are never stuck behind a next-chunk proj that
            # waits on DMA; attn_pv trails by one chunk so the in-order PE
            # queue never waits on exp before the next chunk's proj/S.
            proj(0)
            for j in range(NCHUNK):
                attn_s(j)
                if j + 2 < NCHUNK:
                    load_x(j + 2)
                if j + 1 < NCHUNK:
                    proj(j + 1)
                if j > 0:
                    attn_pv(j - 1)
                    flush_out(j - 1)
            attn_pv(NCHUNK - 1, tail=True)
            flush_out(NCHUNK - 1)
    nc.compile()
    return nc


def _host_mask(p):
    """[128, 2, 256] bf16: [tri|ones] for p=0, [tri|zeros] for p=1."""
    s = np.arange(128)[:, None]
    c = np.arange(128)[None, :]
    tri = (s <= c).astype(np.float32)
    second = np.ones((128, 128), np.float32) if p == 0 else np.zeros((128, 128), np.float32)
    m = np.concatenate([tri, second], axis=1)  # [128, 256]
    return np.concatenate([m, m], axis=1).astype(BF16)  # [128, 512] = [2, 256]


def kernel(x, Wk, bk, Wq, bq, Wv, bv):
    from concourse.bass_utils import run_bass_kernel_spmd

    if "nc" not in _cache:
        _cache["nc"] = _build_program()
    nc = _cache["nc"]

    x = np.asarray(x, np.float32)
    wkq_f = A_W * np.concatenate([np.asarray(Wk), np.asarray(Wq)],
                                 axis=1).astype(np.float32)
    wkq = wkq_f.astype(F8)
    wkqb = (wkq_f / G_XL).astype(F8)
    wkql = (wkq_f - wkq.astype(np.float32)).astype(F8)
    wv_f = np.asarray(Wv, np.float32)
    wv8 = (A_W * wv_f).astype(F8)
    wv8b = (A_W / G_XL * wv_f).astype(F8)
    wvl = (A_W * wv_f - wv8.astype(np.float32)).astype(F8)
    shuf = lambda w: np.ascontiguousarray(
        w.reshape(8, 128, -1).transpose(1, 0, 2).reshape(128, -1))
    wpack = np.concatenate(
        [shuf(w) for w in (wkq, wkqb, wkql, wv8, wv8b, wvl)], axis=1)
    bias = np.zeros((128, 1), np.float32)
    bias[0:64, 0] = KQ_SCALE * np.asarray(bk, np.float32)

    in_maps = []
    for core in range(NCORES):
        b, p = core // 2, core % 2
        xb = x[b]  # [T, D]
        if p == 1:  # pair-swap 128-row blocks within 256-row pairs
            xb = xb.reshape(T // 256, 2, 128, D)[:, ::-1].reshape(T, D)
        xT = np.ascontiguousarray(xb.T)        # [D, T-device]
        x8 = xT.astype(F8)
        xl8 = (G_XL * (xT - x8.astype(np.float32))).astype(F8)
        in_maps.append({
            "x8": x8,
            "xl8": xl8,
            "wpack": wpack,
            "bias": bias,
            "mask": _host_mask(p),
        })

    res = run_bass_kernel_spmd(nc, in_maps, core_ids=list(range(NCORES)))
    results = res.results
    _cache["last_run"] = res

    bv_f = np.asarray(bv, np.float32)
    out = np.zeros((B, T, H), np.float32)
    for b in range(B):
        a0 = results[2 * b]["out"]      # [T-device, 65], device == orig for p=0
        a1 = results[2 * b + 1]["out"]  # [T-device, 65], pair-swapped
        a1 = a1.reshape(T // 256, 2, 128, 65)[:, ::-1].reshape(T, 65)
        tot = a0 + a1
        # v was carried at scale A_W on-device
        out[b] = tot[:, 0:64] / (A_W * tot[:, 64:65]) + bv_f
    return out


# revision 53
# speedup vs baseline: 1.2077x; 1.0367x over previous
"""Trainium2 Bass kernel for nn_AttentionHead_80436147520097.

Single attention head, B=4 T=4096 D=1024 H=64:
    k,q,v = x@W+b;  S[t,s] = k_t . q_s / 8 (causal s<=t);  out = softmax_s(S) @ v

Sharding: 8 cores = 4 batches x 2 parity groups. Within a batch, the two
cores split the softmax (s) dimension by 128-row block parity: core p owns
s-blocks with (block % 2 == p). Parity divergence is pushed into host data
(x columns pair-swapped for p==1, per-parity diag masks), so all 8 cores
run ONE SPMD program. Each core emits partial unnormalized out [t, 65]
(col 64 = softmax denominator) over its s-half; host adds halves, divides,
adds bv, un-permutes.

Device math (bq is dropped: softmax is shift-invariant per t-row since the
final normalization divides by the same-shifted denominator; bv is added on
the host: out = num/den + bv; bk is folded into the fp8 k conversion):

- x is fed as fp8e4m3 (x8) plus an fp8 residual (xl8, own s-columns only).
- kq proj: one fp8 DoubleRow pass [W8k|W8q]^T x8 over full T -> PSUM
  [k;q][128, 512] per chunk -> DVE converts to fp8 (scale 16, +16*bk on k
  rows) giving k8 [64, T] and q8 rows; q8 own-columns are copied by an
  SBUF DMA into the zero-interleaved DR stationary q8z [64, task, 2, 128]
  (i=1 tile is zeros so DoubleRow contributes q8^T k8 only).
- v proj (error-sensitive, needs ~bf16 quality): 3-pass fp8 residual
  (x8@Wv8 + xl8@Wv8 + x8@Wvl8) with x-slices as the STATIONARY so the
  output lands v-natural [s=128, h] directly (no transposes).
- S^T[s-task, t-chunk] = DoubleRow(q8z[task], k8[chunk(+ghost)]) * (0.125/256)
- exp: Activation engine (Exp) for most (pair, chunk) tiles; a tunable
  fraction on DVE via Schraudolph fast-exp (x*c1+c2 -> int16 -> bf16 bits).
- diag masking: per chunk j, pair j is diagonal; e columns [0:256] and
  [768:1024] multiply a per-parity [128, 2, 256] mask ([tri|ones] for p=0,
  [tri|zeros] for p=1) -- uniform across tasks.
- PV flipped: out[t-block, 0:65] += e[s-task, t-block]^T @ [v_task|1]
  (65-wide moving operand), skipping dead blocks (m <= jd//2), PSUM group
  per t-block, DVE drains to SBUF outbuf, DMA out.
"""

import sys

import numpy as np

try:
    import ml_dtypes
except ImportError:  # pragma: no cover
    sys.path.insert(0, "/opt/trn_rl_repo")
    import ml_dtypes

B, T, D, H = 4, 4096, 1024, 64
NCORES = 8
NCHUNK = 8           # t-chunks of 512
NTASK = 16           # own s-tasks (128 rows each)
BF16 = ml_dtypes.bfloat16
F8 = ml_dtypes.float8_e4m3

KQ_SCALE = 16.0      # k8 = 16(k+bk), q8 = 16 q
S_SCALE = 0.125 / (KQ_SCALE * KQ_SCALE)
A_W = 512.0          # fp8 weight pre-scale (keeps W in e4m3 normal range)
G_XL = 16.0          # fp8 x-residual pre-scale
# fraction (in tenths) of exp tiles computed on DVE via Schraudolph
SCHRAUD_TENTHS = 3

_cache = {}


def _build_program():
    import concourse.bacc as bacc
    import concourse.mybir as mybir
    import concourse.tile as tile

    f32 = mybir.dt.float32
    bf16 = mybir.dt.bfloat16
    fp8 = mybir.dt.float8e4
    i16 = mybir.dt.int16
    DR = mybir.MatmulPerfMode.DoubleRow
    Exp = mybir.ActivationFunctionType.Exp

    LOG2E = 1.4426950408889634
    SCH_C1 = S_SCALE * LOG2E * 128.0
    SCH_C2 = 127.0 * 128.0 - 0.5 * 128.0 * 0.0579 + 0.5

    nc = bacc.Bacc("TRN2", target_bir_lowering=False, debug=False,
                   num_devices=NCORES)

    x8_d = nc.dram_tensor("x8", [D, T], fp8, kind="ExternalInput").ap()
    xl8_d = nc.dram_tensor("xl8", [D, T], fp8, kind="ExternalInput").ap()
    wpack_d = nc.dram_tensor("wpack", [128, 4608], fp8, kind="ExternalInput").ap()
    bias_d = nc.dram_tensor("bias", [128, 1], f32, kind="ExternalInput").ap()
    mask_d = nc.dram_tensor("mask", [128, 512], bf16, kind="ExternalInput").ap()
    out_d = nc.dram_tensor("out", [T, 65], f32, kind="ExternalOutput").ap()

    with tile.TileContext(nc) as tc:
        with (
            tc.tile_pool(name="const", bufs=1) as const,
            tc.tile_pool(name="x8p", bufs=1) as x8p,
            tc.tile_pool(name="sb", bufs=1) as sb,
            tc.tile_pool(name="e", bufs=20) as e_p,
            tc.tile_pool(name="proj_ps", bufs=1, space="PSUM") as proj_psp,
            tc.tile_pool(name="s_ps", bufs=2, space="PSUM") as s_psp,
            tc.tile_pool(name="s_ps2", bufs=1, space="PSUM") as s_psp2,
            tc.tile_pool(name="pv_ps", bufs=1, space="PSUM") as pv_psp,
        ):
            # ---------------- constants ----------------
            # one pre-shuffled weight pack: [wkq|wkqb|wkql (1024 each) |
            # wv8|wv8b|wvl (512 each)], all [p, c, m] c-major
            wpack = const.tile([128, 4608], fp8)
            nc.sync.dma_start(wpack[:], wpack_d)
            x8 = x8p.tile([128, 8 * T], fp8)        # [p, c, t-device]
            x83 = x8[:].rearrange("p (c t) -> p c t", c=8)
            x8d3 = x8_d.rearrange("(c p) t -> p c t", p=128)
            xl8 = x8p.tile([128, 8 * T], fp8)       # 16*(x - x8), full T
            xl83 = xl8[:].rearrange("p (c t) -> p c t", c=8)
            xl8d3 = xl8_d.rearrange("(c p) t -> p c t", p=128)
            bias = const.tile([128, 1], f32)
            masks = const.tile([128, 512], bf16)    # [p, 2, 256] per parity

            # ---------------- persistent SBUF ----------------
            # kq8 [128, chunk, 1024]: cols [1024j, 1024j+512) = chunk j
            # (rows 0:64 = k8, 64:128 = q8); cols [+512, +1024) = that
            # chunk's dedicated DR-ghost pad (must be finite, never real)
            kq8 = sb.tile([128, NCHUNK * 1024], fp8)
            kq83 = kq8[:].rearrange("p (j n) -> p j n", j=NCHUNK)
            q8z = sb.tile([64, NTASK * 256], fp8)   # [64, task, 2, 128], i=1 zeros
            # i=1 slots of q8z MUST be 0; pads must be finite. Small separate
            # memsets on gpsimd (idle engine) so nothing stalls on one big op.
            nc.gpsimd.memset(
                q8z[:].rearrange("p (n i m) -> p n i m", n=NTASK, i=2)[:, :, 1:2, :],
                0.0)
            for j in range(NCHUNK):
                nc.gpsimd.memset(kq83[:, j, 512:1024], 0.0)
            v_nat = sb.tile([128, NTASK * 65], bf16)
            ones_col = v_nat[:].rearrange("p (n w) -> p n w", w=65)[:, :, 64:65]
            nc.vector.memset(ones_col, 1.0)
            outbuf = sb.tile([128, 32 * 65], f32)
            # prefetch exp table off the critical path
            scratch = const.tile([1, 8], f32)
            nc.vector.memset(scratch[:], 0.0)
            nc.scalar.activation(scratch[:], scratch[:], Exp)

            q8z4 = q8z[:].rearrange("p (n i m) -> p n i m", n=NTASK, i=2)
            wkq3 = wpack[:, 0:1024].rearrange("p (c m) -> p c m", c=8)
            wkqb3 = wpack[:, 1024:2048].rearrange("p (c m) -> p c m", c=8)
            wkql3 = wpack[:, 2048:3072].rearrange("p (c m) -> p c m", c=8)
            wv83 = wpack[:, 3072:3584].rearrange("p (c m) -> p c m", c=8)
            wv8b3 = wpack[:, 3584:4096].rearrange("p (c m) -> p c m", c=8)
            wvl3 = wpack[:, 4096:4608].rearrange("p (c m) -> p c m", c=8)
            v3 = v_nat[:].rearrange("p (n w) -> p n w", w=65)

            def proj(j):
                t0 = 512 * j
                # --- kq projection: 3 fp8-DR residual passes over chunk j ---
                ps = proj_psp.tile([128, 512], f32, tag="proj")
                for pi, (w3, xs) in enumerate(
                        ((wkq3, x83), (wkql3, x83), (wkqb3, xl83))):
                    for u in range(4):
                        c2 = slice(2 * u, 2 * u + 2)
                        nc.tensor.matmul(
                            ps[:], w3[:, c2, :], xs[:, c2, t0:t0 + 512],
                            start=(pi == 0 and u == 0),
                            stop=(pi == 2 and u == 3),
                            perf_mode=DR, skip_group_check=True)
                # psum holds A_W*[k;q]; convert to k8=16(k+bk), q8=16q
                nc.vector.tensor_scalar(
                    out=kq83[:, j, 0:512], in0=ps[:],
                    scalar1=KQ_SCALE / A_W, scalar2=bias[:, 0:1],
                    op0=mybir.AluOpType.mult, op1=mybir.AluOpType.add)
                # q8z fill: own cols of chunk j (device blocks 0 and 2 of
                # the chunk) -> tasks 2j, 2j+1 slot i=0. stream_shuffle with
                # an identity mask = partition-shifted copy on DVE (cheaper
                # chain than an SBUF DMA: no HWDGE, no DMA-sem latency).
                nc.vector.stream_shuffle(
                    q8z4[:, 2 * j:2 * j + 2, 0, :],
                    kq83[:, j, 0:512][64:128, :]
                    .rearrange("p (b m) -> p b m", b=4)[:, 0:3:2, :],
                    mask=list(range(32)))
                # --- v projection for tasks 2j, 2j+1: 3-pass fp8 residual,
                # x-slice stationary -> v natural [s=128, h] ---
                for dm in range(2):
                    m = 2 * j + dm
                    vp = proj_psp.tile([128, 512], f32, tag="proj")
                    xo = x83[:, :, t0 + 256 * dm: t0 + 256 * dm + 128]
                    xlo = xl83[:, :, t0 + 256 * dm: t0 + 256 * dm + 128]
                    for pi, (xs, w3) in enumerate(
                            ((xo, wv83), (xo, wvl3), (xlo, wv8b3))):
                        for u in range(4):
                            c2 = slice(2 * u, 2 * u + 2)
                            nc.tensor.matmul(
                                vp[:, 0:64], xs[:, c2, :], w3[:, c2, :],
                                start=(pi == 0 and u == 0),
                                stop=(pi == 2 and u == 3),
                                perf_mode=DR, skip_group_check=True)
                    nc.vector.tensor_copy(out=v3[:, m, 0:64], in_=vp[:, 0:64])

            e_tiles = {}

            def attn_s(j):
                # S + exp per pair P = (tasks 2P, 2P+1)
                es = e_tiles[j] = [None] * (j + 1)
                # last chunk: diagonal pair first so its gpsimd mask never
                # sits after the final exp on the critical tail
                order = ([j] + list(range(j))) if j == NCHUNK - 1 else range(j + 1)
                for P in order:
                    pool = s_psp2 if P % 3 == 2 else s_psp
                    ps = pool.tile([128, 1024], f32, tag="s")
                    e = e_p.tile([128, 1024], bf16, tag="e")
                    for h in range(2):
                        nc.tensor.matmul(
                            ps[:, 512 * h:512 * h + 512],
                            q8z4[:, 2 * P + h, :, :],
                            kq83[:, j, :][0:64, :]
                            .rearrange("p (i n) -> p i n", i=2),
                            start=True, stop=True,
                            perf_mode=DR, skip_group_check=True)
                    if (P * 3 + j) % 10 < SCHRAUD_TENTHS and P != j:
                        nc.vector.tensor_scalar(
                            out=e[:].bitcast(i16), in0=ps[:],
                            scalar1=SCH_C1, scalar2=SCH_C2,
                            op0=mybir.AluOpType.mult,
                            op1=mybir.AluOpType.add)
                    else:
                        nc.scalar.activation(e[:], ps[:], Exp, scale=S_SCALE)
                    if P == j:  # diagonal pair: mask cols [0:256], [768:1024]
                        src = e[:].rearrange("p (a n) -> p a n", n=256)[:, 0:4:3, :]
                        nc.gpsimd.tensor_mul(
                            src, src,
                            masks[:].rearrange("p (a n) -> p a n", n=256))
                    es[P] = e

            def pv_group(j, dj, tail=False):
                es = e_tiles[j]
                jd = 4 * j + dj
                if tail:  # S rings are idle after the last exp
                    pool = s_psp2 if dj % 3 == 2 else s_psp
                    po = pool.tile([128, 1024], f32, tag="s")
                else:
                    po = pv_psp.tile([128, 512], f32, tag="pv")
                for m in range(jd // 2 + 1):
                    e = es[m // 2]
                    col = 512 * (m % 2) + 128 * (jd % 4)
                    nc.tensor.matmul(
                        po[:, 0:65],
                        e[:, col:col + 128],
                        v3[:, m, :],
                        start=(m == 0), stop=(m == jd // 2),
                        skip_group_check=True)
                nc.vector.tensor_copy(
                    out=outbuf[:, 65 * jd:65 * jd + 65],
                    in_=po[:, 0:65])

            def attn_pv(j, tail=False):
                for dj in range(4):
                    pv_group(j, dj, tail=tail)
                e_tiles.pop(j)

            out_r = out_d.rearrange("(jd t) h -> t jd h", t=128)
            def load_x(j):
                # split by c-halves so the first proj matmuls can start
                # after a quarter of the chunk's bytes have landed
                for ch in range(2):
                    cs = slice(4 * ch, 4 * ch + 4)
                    nc.sync.dma_start(x83[:, cs, 512 * j:512 * (j + 1)],
                                      x8d3[:, cs, 512 * j:512 * (j + 1)])
                    nc.sync.dma_start(xl83[:, cs, 512 * j:512 * (j + 1)],
                                      xl8d3[:, cs, 512 * j:512 * (j + 1)])

            def flush_out(j):
                if j % 2 == 1:  # flush 8 t-blocks
                    g = j // 2
                    nc.sync.dma_start(
                        out_r[:, 8 * g:8 * (g + 1), :],
                        outbuf[:, 520 * g:520 * (g + 1)]
                        .rearrange("p (jd h) -> p jd h", h=65))

            load_x(0)
            nc.sync.dma_start(bias[:], bias_d)
            nc.sync.dma_start(masks[:], mask_d)
            load_x(1)
            # software-pipelined: attn_s(j) issues right after proj(j) so
            # the first S/exp are never stuck behind a next-chunk proj that
            # waits on DMA; attn_pv trails by one chunk so the in-order PE
            # queue never waits on exp before the next chunk's proj/S.
            proj(0)
            for j in range(NCHUNK):
                attn_s(j)
                if j + 2 < NCHUNK:
                    load_x(j + 2)
                if j + 1 < NCHUNK:
                    proj(j + 1)
                if j > 0:
                    attn_pv(j - 1)
                    flush_out(j - 1)
            attn_pv(NCHUNK - 1, tail=True)
            flush_out(NCHUNK - 1)
    nc.compile()
    return nc


def _host_mask(p):
    """[128, 2, 256] bf16: [tri|ones] for p=0, [tri|zeros] for p=1."""
    s = np.arange(128)[:, None]
    c = np.arange(128)[None, :]
    tri = (s <= c).astype(np.float32)
    second = np.ones((128, 128), np.float32) if p == 0 else np.zeros((128, 128), np.float32)
    m = np.concatenate([tri, second], axis=1)  # [128, 256]
    return np.concatenate([m, m], axis=1).astype(BF16)  # [128, 512] = [2, 256]


def kernel(x, Wk, bk, Wq, bq, Wv, bv):
    from concourse.bass_utils import run_bass_kernel_spmd

    if "nc" not in _cache:
        _cache["nc"] = _build_program()
    nc = _cache["nc"]

    x = np.asarray(x, np.float32)
    wkq_f = A_W * np.concatenate([np.asarray(Wk), np.asarray(Wq)],
                                 axis=1).astype(np.float32)
    wkq = wkq_f.astype(F8)
    wkqb = (wkq_f / G_XL).astype(F8)
    wkql = (wkq_f - wkq.astype(np.float32)).astype(F8)
    wv_f = np.asarray(Wv, np.float32)
    wv8 = (A_W * wv_f).astype(F8)
    wv8b = (A_W / G_XL * wv_f).astype(F8)
    wvl = (A_W * wv_f - wv8.astype(np.float32)).astype(F8)
    shuf = lambda w: np.ascontiguousarray(
        w.reshape(8, 128, -1).transpose(1, 0, 2).reshape(128, -1))
    wpack = np.concatenate(
        [shuf(w) for w in (wkq, wkqb, wkql, wv8, wv8b, wvl)], axis=1)
    bias = np.zeros((128, 1), np.float32)
    bias[0:64, 0] = KQ_SCALE * np.asarray(bk, np.float32)

    in_maps = []
    for core in range(NCORES):
        b, p = core // 2, core % 2
        xb = x[b]  # [T, D]
        if p == 1:  # pair-swap 128-row blocks within 256-row pairs
            xb = xb.reshape(T // 256, 2, 128, D)[:, ::-1].reshape(T, D)
        xT = np.ascontiguousarray(xb.T)        # [D, T-device]
        x8 = xT.astype(F8)
        xl8 = (G_XL * (xT - x8.astype(np.float32))).astype(F8)
        in_maps.append({
            "x8": x8,
            "xl8": xl8,
            "wpack": wpack,
            "bias": bias,
            "mask": _host_mask(p),
        })

    res = run_bass_kernel_spmd(nc, in_maps, core_ids=list(range(NCORES)))
    results = res.results
    _cache["last_run"] = res

    bv_f = np.asarray(bv, np.float32)
    out = np.zeros((B, T, H), np.float32)
    for b in range(B):
        a0 = results[2 * b]["out"]      # [T-device, 65], device == orig for p=0
        a1 = results[2 * b + 1]["out"]  # [T-device, 65], pair-swapped
        a1 = a1.reshape(T // 256, 2, 128, 65)[:, ::-1].reshape(T, 65)
        tot = a0 + a1
        # v was carried at scale A_W on-device
        out[b] = tot[:, 0:64] / (A_W * tot[:, 64:65]) + bv_f
    return out


# revision 59
# speedup vs baseline: 1.2266x; 1.0157x over previous
"""Trainium2 Bass kernel for nn_AttentionHead_80436147520097.

Single attention head, B=4 T=4096 D=1024 H=64:
    k,q,v = x@W+b;  S[t,s] = k_t . q_s / 8 (causal s<=t);  out = softmax_s(S) @ v

Sharding: 8 cores = 4 batches x 2 parity groups. Within a batch, the two
cores split the softmax (s) dimension by 128-row block parity: core p owns
s-blocks with (block % 2 == p). Parity divergence is pushed into host data
(x columns pair-swapped for p==1, per-parity diag masks), so all 8 cores
run ONE SPMD program. Each core emits partial unnormalized out [t, 65]
(col 64 = softmax denominator) over its s-half; host adds halves, divides,
adds bv, un-permutes.

Device math (bq is dropped: softmax is shift-invariant per t-row since the
final normalization divides by the same-shifted denominator; bv is added on
the host: out = num/den + bv; bk is folded into the fp8 k conversion):

- x is fed as fp8e4m3 (x8) plus an fp8 residual (xl8, own s-columns only).
- kq proj: one fp8 DoubleRow pass [W8k|W8q]^T x8 over full T -> PSUM
  [k;q][128, 512] per chunk -> DVE converts to fp8 (scale 16, +16*bk on k
  rows) giving k8 [64, T] and q8 rows; q8 own-columns are copied by an
  SBUF DMA into the zero-interleaved DR stationary q8z [64, task, 2, 128]
  (i=1 tile is zeros so DoubleRow contributes q8^T k8 only).
- v proj (error-sensitive, needs ~bf16 quality): 3-pass fp8 residual
  (x8@Wv8 + xl8@Wv8 + x8@Wvl8) with x-slices as the STATIONARY so the
  output lands v-natural [s=128, h] directly (no transposes).
- S^T[s-task, t-chunk] = DoubleRow(q8z[task], k8[chunk(+ghost)]) * (0.125/256)
- exp: Activation engine (Exp) for most (pair, chunk) tiles; a tunable
  fraction on DVE via Schraudolph fast-exp (x*c1+c2 -> int16 -> bf16 bits).
- diag masking: per chunk j, pair j is diagonal; e columns [0:256] and
  [768:1024] multiply a per-parity [128, 2, 256] mask ([tri|ones] for p=0,
  [tri|zeros] for p=1) -- uniform across tasks.
- PV flipped: out[t-block, 0:65] += e[s-task, t-block]^T @ [v_task|1]
  (65-wide moving operand), skipping dead blocks (m <= jd//2), PSUM group
  per t-block, DVE drains to SBUF outbuf, DMA out.
"""

import sys

import numpy as np

try:
    import ml_dtypes
except ImportError:  # pragma: no cover
    sys.path.insert(0, "/opt/trn_rl_repo")
    import ml_dtypes

B, T, D, H = 4, 4096, 1024, 64
NCORES = 8
NCHUNK = 8           # t-chunks of 512
NTASK = 16           # own s-tasks (128 rows each)
BF16 = ml_dtypes.bfloat16
F8 = ml_dtypes.float8_e4m3

KQ_SCALE = 16.0      # k8 = 16(k+bk), q8 = 16 q
S_SCALE = 0.125 / (KQ_SCALE * KQ_SCALE)
A_W = 512.0          # fp8 weight pre-scale (keeps W in e4m3 normal range)
G_XL = 16.0          # fp8 x-residual pre-scale
# fraction (in tenths) of exp tiles computed on DVE via Schraudolph
SCHRAUD_TENTHS = 3

_cache = {}


def _build_program():
    import concourse.bacc as bacc
    import concourse.mybir as mybir
    import concourse.tile as tile

    f32 = mybir.dt.float32
    bf16 = mybir.dt.bfloat16
    fp8 = mybir.dt.float8e4
    i16 = mybir.dt.int16
    DR = mybir.MatmulPerfMode.DoubleRow
    Exp = mybir.ActivationFunctionType.Exp

    LOG2E = 1.4426950408889634
    SCH_C1 = S_SCALE * LOG2E * 128.0
    SCH_C2 = 127.0 * 128.0 - 0.5 * 128.0 * 0.0579 + 0.5

    nc = bacc.Bacc("TRN2", target_bir_lowering=False, debug=False,
                   num_devices=NCORES)

    xall_d = nc.dram_tensor("xall", [2 * D, T], fp8, kind="ExternalInput").ap()
    wpack_d = nc.dram_tensor("wpack", [128, 4608], fp8, kind="ExternalInput").ap()
    bias_d = nc.dram_tensor("bias", [128, 1], f32, kind="ExternalInput").ap()
    mask_d = nc.dram_tensor("mask", [128, 512], bf16, kind="ExternalInput").ap()
    out_d = nc.dram_tensor("out", [T, 65], f32, kind="ExternalOutput").ap()

    with tile.TileContext(nc) as tc:
        with (
            tc.tile_pool(name="const", bufs=1) as const,
            tc.tile_pool(name="x8p", bufs=1) as x8p,
            tc.tile_pool(name="sb", bufs=1) as sb,
            tc.tile_pool(name="e", bufs=20) as e_p,
            tc.tile_pool(name="proj_ps", bufs=1, space="PSUM") as proj_psp,
            tc.tile_pool(name="s_ps", bufs=2, space="PSUM") as s_psp,
            tc.tile_pool(name="s_ps2", bufs=1, space="PSUM") as s_psp2,
            tc.tile_pool(name="pv_ps", bufs=1, space="PSUM") as pv_psp,
        ):
            # ---------------- constants ----------------
            # one pre-shuffled weight pack: [wkq|wkqb|wkql (1024 each) |
            # wv8|wv8b|wvl (512 each)], all [p, c, m] c-major
            wpack = const.tile([128, 4608], fp8)
            nc.sync.dma_start(wpack[:], wpack_d)
            # [p, c, t]: c 0-7 = x8, c 8-15 = 16*(x - x8)
            xall = x8p.tile([128, 16 * T], fp8)
            xall3 = xall[:].rearrange("p (c t) -> p c t", c=16)
            xalld3 = xall_d.rearrange("(c p) t -> p c t", p=128)
            x83 = xall3[:, 0:8, :]
            xl83 = xall3[:, 8:16, :]
            bias = const.tile([128, 1], f32)
            masks = const.tile([128, 512], bf16)    # [p, 2, 256] per parity

            # ---------------- persistent SBUF ----------------
            # kq8 [128, chunk, 1024]: cols [1024j, 1024j+512) = chunk j
            # (rows 0:64 = k8, 64:128 = q8); cols [+512, +1024) = that
            # chunk's dedicated DR-ghost pad (must be finite, never real)
            kq8 = sb.tile([128, NCHUNK * 1024], fp8)
            kq83 = kq8[:].rearrange("p (j n) -> p j n", j=NCHUNK)
            q8z = sb.tile([64, NTASK * 256], fp8)   # [64, task, 2, 128], i=1 zeros
            # i=1 slots of q8z MUST be 0; pads must be finite. Small separate
            # memsets on gpsimd (idle engine) so nothing stalls on one big op.
            nc.gpsimd.memset(
                q8z[:].rearrange("p (n i m) -> p n i m", n=NTASK, i=2)[:, :, 1:2, :],
                0.0)
            for j in range(NCHUNK):
                nc.gpsimd.memset(kq83[:, j, 512:1024], 0.0)
            v_nat = sb.tile([128, NTASK * 65], bf16)
            ones_col = v_nat[:].rearrange("p (n w) -> p n w", w=65)[:, :, 64:65]
            nc.vector.memset(ones_col, 1.0)
            outbuf = sb.tile([128, 32 * 65], f32)
            # prefetch exp table off the critical path
            scratch = const.tile([1, 8], f32)
            nc.vector.memset(scratch[:], 0.0)
            nc.scalar.activation(scratch[:], scratch[:], Exp)


            q8z4 = q8z[:].rearrange("p (n i m) -> p n i m", n=NTASK, i=2)
            wkq3 = wpack[:, 0:1024].rearrange("p (c m) -> p c m", c=8)
            wkqb3 = wpack[:, 1024:2048].rearrange("p (c m) -> p c m", c=8)
            wkql3 = wpack[:, 2048:3072].rearrange("p (c m) -> p c m", c=8)
            wv83 = wpack[:, 3072:3584].rearrange("p (c m) -> p c m", c=8)
            wv8b3 = wpack[:, 3584:4096].rearrange("p (c m) -> p c m", c=8)
            wvl3 = wpack[:, 4096:4608].rearrange("p (c m) -> p c m", c=8)
            v3 = v_nat[:].rearrange("p (n w) -> p n w", w=65)

            def proj(j):
                t0 = 512 * j
                # --- kq projection: 3 fp8-DR residual passes over chunk j ---
                ps = proj_psp.tile([128, 512], f32, tag="proj")
                for pi, (w3, xs) in enumerate(
                        ((wkq3, x83), (wkql3, x83), (wkqb3, xl83))):
                    for u in range(4):
                        c2 = slice(2 * u, 2 * u + 2)
                        nc.tensor.matmul(
                            ps[:], w3[:, c2, :], xs[:, c2, t0:t0 + 512],
                            start=(pi == 0 and u == 0),
                            stop=(pi == 2 and u == 3),
                            perf_mode=DR, skip_group_check=True)
                # psum holds A_W*[k;q]; convert to k8=16(k+bk), q8=16q
                nc.vector.tensor_scalar(
                    out=kq83[:, j, 0:512], in0=ps[:],
                    scalar1=KQ_SCALE / A_W, scalar2=bias[:, 0:1],
                    op0=mybir.AluOpType.mult, op1=mybir.AluOpType.add)
                # q8z fill: own cols of chunk j (device blocks 0 and 2 of
                # the chunk) -> tasks 2j, 2j+1 slot i=0. stream_shuffle with
                # an identity mask = partition-shifted copy on DVE (cheaper
                # chain than an SBUF DMA: no HWDGE, no DMA-sem latency).
                nc.vector.stream_shuffle(
                    q8z4[:, 2 * j:2 * j + 2, 0, :],
                    kq83[:, j, 0:512][64:128, :]
                    .rearrange("p (b m) -> p b m", b=4)[:, 0:3:2, :],
                    mask=list(range(32)))
                # --- v projection for tasks 2j, 2j+1: 3-pass fp8 residual,
                # x-slice stationary -> v natural [s=128, h] ---
                for dm in range(2):
                    m = 2 * j + dm
                    vp = proj_psp.tile([128, 512], f32, tag="proj")
                    xo = x83[:, :, t0 + 256 * dm: t0 + 256 * dm + 128]
                    xlo = xl83[:, :, t0 + 256 * dm: t0 + 256 * dm + 128]
                    for pi, (xs, w3) in enumerate(
                            ((xo, wv83), (xo, wvl3), (xlo, wv8b3))):
                        for u in range(4):
                            c2 = slice(2 * u, 2 * u + 2)
                            nc.tensor.matmul(
                                vp[:, 0:64], xs[:, c2, :], w3[:, c2, :],
                                start=(pi == 0 and u == 0),
                                stop=(pi == 2 and u == 3),
                                perf_mode=DR, skip_group_check=True)
                    nc.vector.tensor_copy(out=v3[:, m, 0:64], in_=vp[:, 0:64])

            e_tiles = {}

            def attn_s(j):
                # S + exp per pair P = (tasks 2P, 2P+1)
                es = e_tiles[j] = [None] * (j + 1)
                # last chunk: diagonal pair first so its gpsimd mask never
                # sits after the final exp on the critical tail
                order = ([j] + list(range(j))) if j == NCHUNK - 1 else range(j + 1)
                for P in order:
                    pool = s_psp2 if P % 3 == 2 else s_psp
                    ps = pool.tile([128, 1024], f32, tag="s")
                    e = e_p.tile([128, 1024], bf16, tag="e")
                    for h in range(2):
                        nc.tensor.matmul(
                            ps[:, 512 * h:512 * h + 512],
                            q8z4[:, 2 * P + h, :, :],
                            kq83[:, j, :][0:64, :]
                            .rearrange("p (i n) -> p i n", i=2),
                            start=True, stop=True,
                            perf_mode=DR, skip_group_check=True)
                    if (P * 3 + j) % 10 < SCHRAUD_TENTHS and P != j:
                        nc.vector.tensor_scalar(
                            out=e[:].bitcast(i16), in0=ps[:],
                            scalar1=SCH_C1, scalar2=SCH_C2,
                            op0=mybir.AluOpType.mult,
                            op1=mybir.AluOpType.add)
                    else:
                        nc.scalar.activation(e[:], ps[:], Exp, scale=S_SCALE)
                    if P == j:  # diagonal pair: mask cols [0:256], [768:1024]
                        src = e[:].rearrange("p (a n) -> p a n", n=256)[:, 0:4:3, :]
                        nc.gpsimd.tensor_mul(
                            src, src,
                            masks[:].rearrange("p (a n) -> p a n", n=256))
                    es[P] = e

            def pv_group(j, dj, tail=False):
                es = e_tiles[j]
                jd = 4 * j + dj
                if tail:  # S rings are idle after the last exp
                    pool = s_psp2 if dj % 3 == 2 else s_psp
                    po = pool.tile([128, 1024], f32, tag="s")
                else:
                    po = pv_psp.tile([128, 512], f32, tag="pv")
                for m in range(jd // 2 + 1):
                    e = es[m // 2]
                    col = 512 * (m % 2) + 128 * (jd % 4)
                    nc.tensor.matmul(
                        po[:, 0:65],
                        e[:, col:col + 128],
                        v3[:, m, :],
                        start=(m == 0), stop=(m == jd // 2),
                        skip_group_check=True)
                nc.vector.tensor_copy(
                    out=outbuf[:, 65 * jd:65 * jd + 65],
                    in_=po[:, 0:65])

            def attn_pv(j, tail=False):
                for dj in range(4):
                    pv_group(j, dj, tail=tail)
                e_tiles.pop(j)

            out_r = out_d.rearrange("(jd t) h -> t jd h", t=128)
            def load_x(j):
                # x8 first (feeds proj passes 1-2), then the xl8 residual
                # (pass 3): one DMA each -> one DMA-sem wait per pass group
                for ch in range(2):
                    cs = slice(8 * ch, 8 * ch + 8)
                    nc.sync.dma_start(xall3[:, cs, 512 * j:512 * (j + 1)],
                                      xalld3[:, cs, 512 * j:512 * (j + 1)])

            def flush_out(j):
                # flush this chunk's 4 t-blocks
                nc.sync.dma_start(
                    out_r[:, 4 * j:4 * (j + 1), :],
                    outbuf[:, 260 * j:260 * (j + 1)]
                    .rearrange("p (jd h) -> p jd h", h=65))

            load_x(0)
            nc.sync.dma_start(bias[:], bias_d)
            nc.sync.dma_start(masks[:], mask_d)
            load_x(1)
            # software-pipelined: attn_s(j) issues right after proj(j) so
            # the first S/exp are never stuck behind a next-chunk proj that
            # waits on DMA; attn_pv trails by one chunk so the in-order PE
            # queue never waits on exp before the next chunk's proj/S.
            proj(0)
            for j in range(NCHUNK):
                attn_s(j)
                if j + 2 < NCHUNK:
                    load_x(j + 2)
                if j + 1 < NCHUNK:
                    proj(j + 1)
                if j > 0:
                    attn_pv(j - 1)
                    flush_out(j - 1)
            attn_pv(NCHUNK - 1, tail=True)
            flush_out(NCHUNK - 1)
    nc.compile()
    return nc


def _host_mask(p):
    """[128, 2, 256] bf16: [tri|ones] for p=0, [tri|zeros] for p=1."""
    s = np.arange(128)[:, None]
    c = np.arange(128)[None, :]
    tri = (s <= c).astype(np.float32)
    second = np.ones((128, 128), np.float32) if p == 0 else np.zeros((128, 128), np.float32)
    m = np.concatenate([tri, second], axis=1)  # [128, 256]
    return np.concatenate([m, m], axis=1).astype(BF16)  # [128, 512] = [2, 256]


def kernel(x, Wk, bk, Wq, bq, Wv, bv):
    from concourse.bass_utils import run_bass_kernel_spmd

    if "nc" not in _cache:
        _cache["nc"] = _build_program()
    nc = _cache["nc"]

    x = np.asarray(x, np.float32)
    wkq_f = A_W * np.concatenate([np.asarray(Wk), np.asarray(Wq)],
                                 axis=1).astype(np.float32)
    wkq = wkq_f.astype(F8)
    wkqb = (wkq_f / G_XL).astype(F8)
    wkql = (wkq_f - wkq.astype(np.float32)).astype(F8)
    wv_f = np.asarray(Wv, np.float32)
    wv8 = (A_W * wv_f).astype(F8)
    wv8b = (A_W / G_XL * wv_f).astype(F8)
    wvl = (A_W * wv_f - wv8.astype(np.float32)).astype(F8)
    shuf = lambda w: np.ascontiguousarray(
        w.reshape(8, 128, -1).transpose(1, 0, 2).reshape(128, -1))
    wpack = np.concatenate(
        [shuf(w) for w in (wkq, wkqb, wkql, wv8, wv8b, wvl)], axis=1)
    bias = np.zeros((128, 1), np.float32)
    bias[0:64, 0] = KQ_SCALE * np.asarray(bk, np.float32)

    in_maps = []
    for core in range(NCORES):
        b, p = core // 2, core % 2
        xb = x[b]  # [T, D]
        if p == 1:  # pair-swap 128-row blocks within 256-row pairs
            xb = xb.reshape(T // 256, 2, 128, D)[:, ::-1].reshape(T, D)
        xT = np.ascontiguousarray(xb.T)        # [D, T-device]
        x8 = xT.astype(F8)
        xl8 = (G_XL * (xT - x8.astype(np.float32))).astype(F8)
        in_maps.append({
            "xall": np.ascontiguousarray(np.concatenate([x8, xl8], axis=0)),
            "wpack": wpack,
            "bias": bias,
            "mask": _host_mask(p),
        })

    res = run_bass_kernel_spmd(nc, in_maps, core_ids=list(range(NCORES)))
    results = res.results
    _cache["last_run"] = res

    bv_f = np.asarray(bv, np.float32)
    out = np.zeros((B, T, H), np.float32)
    for b in range(B):
        a0 = results[2 * b]["out"]      # [T-device, 65], device == orig for p=0
        a1 = results[2 * b + 1]["out"]  # [T-device, 65], pair-swapped
        a1 = a1.reshape(T // 256, 2, 128, 65)[:, ::-1].reshape(T, 65)
        tot = a0 + a1
        # v was carried at scale A_W on-device
        out[b] = tot[:, 0:64] / (A_W * tot[:, 64:65]) + bv_f
    return out


# revision 73
# speedup vs baseline: 1.2702x; 1.0356x over previous
"""Trainium2 Bass kernel for nn_AttentionHead_80436147520097.

Single attention head, B=4 T=4096 D=1024 H=64:
    k,q,v = x@W+b;  S[t,s] = k_t . q_s / 8 (causal s<=t);  out = softmax_s(S) @ v

Sharding: 8 cores = 4 batches x 2 parity groups. Within a batch, the two
cores split the softmax (s) dimension by 128-row block parity: core p owns
s-blocks with (block % 2 == p). Parity divergence is pushed into host data
(x columns pair-swapped for p==1, per-parity diag masks), so all 8 cores
run ONE SPMD program. Each core emits partial unnormalized out [t, 65]
(col 64 = softmax denominator) over its s-half; host adds halves, divides,
adds bv, un-permutes.

Device math (bq is dropped: softmax is shift-invariant per t-row since the
final normalization divides by the same-shifted denominator; bv is added on
the host: out = num/den + bv; bk is folded into the fp8 k conversion):

- x is fed as one stacked fp8e4m3 tensor: rows 0:D = x8 = F(x), rows
  D:2D = xl8 = F(16(x - x8)) (full T), 2 DMAs per 512-t chunk.
- kq proj: 3 fp8-DoubleRow residual passes over full T per chunk
  (F(512W)@x8 + F(512W - .)@x8 + F(32W)@xl8 -> PSUM = 512[k;q]) -> DVE
  tensor_scalar converts to fp8 (x1/32, +16*bk on k rows) -> kq8
  [128, chunk, 1024] (cols 512:1024 = per-chunk DR-ghost pad); q8 own
  columns partition-shift to the zero-interleaved DR stationary q8z
  [64, task, 2, 128] via DVE stream_shuffle (identity mask).
- v proj (error-sensitive, needs ~bf16 quality): 3-pass fp8 residual
  with x-slices as the STATIONARY so the output lands v-natural
  [s=128, h] directly (no transposes); v carried at scale 512.
- S^T[s-task, t-chunk] = DoubleRow(q8z[task], kq8[chunk|ghost]), exp
  scale 0.125/256.
- exp: ACT (Exp) for ~70% of (pair, chunk) tiles; the rest on DVE via
  Schraudolph fast-exp (S*c1+c2 -> int16 -> bf16 bits, ~1% rms).
- diag masking: pair j of chunk j; e cols [0:256] and [768:1024]
  multiply a per-parity [128, 2, 256] mask ([tri|ones] p=0,
  [tri|zeros] p=1) on gpsimd -- uniform across tasks.
- PV flipped: out[t-block, 0:65] += e[s-task, t-block]^T @ [v_task|1]
  (65-wide moving operand), skipping dead blocks (m <= jd//2), one PSUM
  group per t-block, DVE drains to SBUF outbuf, per-chunk DMA out.

Schedule: software-pipelined over chunks j: attn_s(j) [S+exp], x-DMA
prefetch (j+2), proj(j+1) [so its DVE convert isn't queued behind exp
work], attn_pv(j-1) [so the in-order PE queue never waits on exp before
the next chunk's proj/S]. PSUM: proj 1 bank, S-pairs 3x[128,1024]
(2 pools), PV 1 bank; the last chunk's PV reuses the idle S rings.
TimelineSim: 53734 ns/core (baseline 76719).
"""

import sys

import numpy as np

try:
    import ml_dtypes
except ImportError:  # pragma: no cover
    sys.path.insert(0, "/opt/trn_rl_repo")
    import ml_dtypes

B, T, D, H = 4, 4096, 1024, 64
NCORES = 8
NCHUNK = 8           # t-chunks of 512
NTASK = 16           # own s-tasks (128 rows each)
BF16 = ml_dtypes.bfloat16
F8 = ml_dtypes.float8_e4m3

KQ_SCALE = 16.0      # k8 = 16(k+bk), q8 = 16 q
S_SCALE = 0.125 / (KQ_SCALE * KQ_SCALE)
A_W = 512.0          # fp8 weight pre-scale (keeps W in e4m3 normal range)
G_XL = 16.0          # fp8 x-residual pre-scale
# fraction (in tenths) of exp tiles computed on DVE via Schraudolph
SCHRAUD_TENTHS = 3
JCUT = 3             # chunks < JCUT skip the kq x-residual pass (their xl8
                     # full-chunk loads are dropped from the DMA-bound ramp)

_cache = {}


def _build_program():
    import concourse.bacc as bacc
    import concourse.mybir as mybir
    import concourse.tile as tile

    f32 = mybir.dt.float32
    bf16 = mybir.dt.bfloat16
    fp8 = mybir.dt.float8e4
    i16 = mybir.dt.int16
    DR = mybir.MatmulPerfMode.DoubleRow
    Exp = mybir.ActivationFunctionType.Exp

    LOG2E = 1.4426950408889634
    SCH_C1 = S_SCALE * LOG2E * 128.0
    SCH_C2 = 127.0 * 128.0 - 0.5 * 128.0 * 0.0579 + 0.5

    nc = bacc.Bacc("TRN2", target_bir_lowering=False, debug=False,
                   num_devices=NCORES)

    xall_d = nc.dram_tensor("xall", [2 * D, T], fp8, kind="ExternalInput").ap()
    xlo_d = nc.dram_tensor("xlo", [D, T // 2], fp8, kind="ExternalInput").ap()
    wpack_d = nc.dram_tensor("wpack", [128, 4608], fp8, kind="ExternalInput").ap()
    bias_d = nc.dram_tensor("bias", [128, 1], f32, kind="ExternalInput").ap()
    mask_d = nc.dram_tensor("mask", [128, 512], bf16, kind="ExternalInput").ap()
    out_d = nc.dram_tensor("out", [T, 65], bf16, kind="ExternalOutput").ap()

    with tile.TileContext(nc) as tc:
        with (
            tc.tile_pool(name="const", bufs=1) as const,
            tc.tile_pool(name="x8p", bufs=1) as x8p,
            tc.tile_pool(name="sb", bufs=1) as sb,
            tc.tile_pool(name="e", bufs=20) as e_p,
            tc.tile_pool(name="proj_ps", bufs=1, space="PSUM") as proj_psp,
            tc.tile_pool(name="s_ps", bufs=2, space="PSUM") as s_psp,
            tc.tile_pool(name="s_ps2", bufs=1, space="PSUM") as s_psp2,
            tc.tile_pool(name="pv_ps", bufs=1, space="PSUM") as pv_psp,
        ):
            # ---------------- constants ----------------
            # one pre-shuffled weight pack: [wkq|wkqb|wkql (1024 each) |
            # wv8|wv8b|wvl (512 each)], all [p, c, m] c-major
            wpack = const.tile([128, 4608], fp8)
            nc.sync.dma_start(wpack[:], wpack_d)
            # [p, c, t]: c 0-7 = x8, c 8-15 = 16*(x - x8)
            xall = x8p.tile([128, 16 * T], fp8)
            xall3 = xall[:].rearrange("p (c t) -> p c t", c=16)
            xalld3 = xall_d.rearrange("(c p) t -> p c t", p=128)
            x83 = xall3[:, 0:8, :]
            xl83 = xall3[:, 8:16, :]
            # xl8 own-columns only (v residual pass), contiguously packed
            xlo = x8p.tile([128, 8 * (T // 2)], fp8)
            xlo3 = xlo[:].rearrange("p (c t) -> p c t", c=8)
            xlod3 = xlo_d.rearrange("(c p) t -> p c t", p=128)
            bias = const.tile([128, 1], f32)
            masks = const.tile([128, 512], bf16)    # [p, 2, 256] per parity

            # ---------------- persistent SBUF ----------------
            # kq8 [128, chunk, 1024]: cols [1024j, 1024j+512) = chunk j
            # (rows 0:64 = k8, 64:128 = q8); cols [+512, +1024) = that
            # chunk's dedicated DR-ghost pad (must be finite, never real)
            kq8 = sb.tile([128, NCHUNK * 1024], fp8)
            kq83 = kq8[:].rearrange("p (j n) -> p j n", j=NCHUNK)
            q8z = sb.tile([64, NTASK * 256], fp8)   # [64, task, 2, 128], i=1 zeros
            # i=1 slots of q8z MUST be 0; pads must be finite. Small separate
            # memsets on gpsimd (idle engine) so nothing stalls on one big op.
            nc.gpsimd.memset(
                q8z[:].rearrange("p (n i m) -> p n i m", n=NTASK, i=2)[:, :, 1:2, :],
                0.0)
            for j in range(NCHUNK):
                nc.gpsimd.memset(kq83[:, j, 512:1024], 0.0)
            v_nat = sb.tile([128, NTASK * 65], bf16)
            ones_col = v_nat[:].rearrange("p (n w) -> p n w", w=65)[:, :, 64:65]
            nc.vector.memset(ones_col, 1.0)
            outbuf = sb.tile([128, 32 * 65], bf16)
            # prefetch exp table off the critical path
            scratch = const.tile([1, 8], f32)
            nc.vector.memset(scratch[:], 0.0)
            nc.scalar.activation(scratch[:], scratch[:], Exp)


            q8z4 = q8z[:].rearrange("p (n i m) -> p n i m", n=NTASK, i=2)
            wkq3 = wpack[:, 0:1024].rearrange("p (c m) -> p c m", c=8)
            wkqb3 = wpack[:, 1024:2048].rearrange("p (c m) -> p c m", c=8)
            wkql3 = wpack[:, 2048:3072].rearrange("p (c m) -> p c m", c=8)
            wv83 = wpack[:, 3072:3584].rearrange("p (c m) -> p c m", c=8)
            wv8b3 = wpack[:, 3584:4096].rearrange("p (c m) -> p c m", c=8)
            wvl3 = wpack[:, 4096:4608].rearrange("p (c m) -> p c m", c=8)
            v3 = v_nat[:].rearrange("p (n w) -> p n w", w=65)

            def proj(j):
                t0 = 512 * j
                # --- kq projection: 3 fp8-DR residual passes over chunk j ---
                ps = proj_psp.tile([128, 512], f32, tag="proj")
                passes = [(wkq3, x83), (wkql3, x83)]
                if j >= JCUT:  # early chunks skip the kq x-residual pass
                    passes.append((wkqb3, xl83))
                for pi, (w3, xs) in enumerate(passes):
                    for u in range(4):
                        c2 = slice(2 * u, 2 * u + 2)
                        nc.tensor.matmul(
                            ps[:], w3[:, c2, :], xs[:, c2, t0:t0 + 512],
                            start=(pi == 0 and u == 0),
                            stop=(pi == len(passes) - 1 and u == 3),
                            perf_mode=DR, skip_group_check=True)
                # psum holds A_W*[k;q]; convert to k8=16(k+bk), q8=16q
                nc.vector.tensor_scalar(
                    out=kq83[:, j, 0:512], in0=ps[:],
                    scalar1=KQ_SCALE / A_W, scalar2=bias[:, 0:1],
                    op0=mybir.AluOpType.mult, op1=mybir.AluOpType.add)
                # q8z fill: own cols of chunk j (device blocks 0 and 2 of
                # the chunk) -> tasks 2j, 2j+1 slot i=0. stream_shuffle with
                # an identity mask = partition-shifted copy on DVE (cheaper
                # chain than an SBUF DMA: no HWDGE, no DMA-sem latency).
                nc.vector.stream_shuffle(
                    q8z4[:, 2 * j:2 * j + 2, 0, :],
                    kq83[:, j, 0:512][64:128, :]
                    .rearrange("p (b m) -> p b m", b=4)[:, 0:3:2, :],
                    mask=list(range(32)))
                # --- v projection for tasks 2j, 2j+1: 3-pass fp8 residual,
                # x-slice stationary -> v natural [s=128, h] ---
                for dm in range(2):
                    m = 2 * j + dm
                    vp = proj_psp.tile([128, 512], f32, tag="proj")
                    xo = x83[:, :, t0 + 256 * dm: t0 + 256 * dm + 128]
                    xro = xlo3[:, :, 128 * m:128 * m + 128]
                    for pi, (xs, w3) in enumerate(
                            ((xo, wv83), (xo, wvl3), (xro, wv8b3))):
                        for u in range(4):
                            c2 = slice(2 * u, 2 * u + 2)
                            nc.tensor.matmul(
                                vp[:, 0:64], xs[:, c2, :], w3[:, c2, :],
                                start=(pi == 0 and u == 0),
                                stop=(pi == 2 and u == 3),
                                perf_mode=DR, skip_group_check=True)
                    nc.vector.tensor_copy(out=v3[:, m, 0:64], in_=vp[:, 0:64])

            e_tiles = {}

            def attn_s(j):
                # S + exp per pair P = (tasks 2P, 2P+1)
                es = e_tiles[j] = [None] * (j + 1)
                # last chunk: diagonal pair first so its gpsimd mask never
                # sits after the final exp on the critical tail
                order = ([j] + list(range(j))) if j == NCHUNK - 1 else range(j + 1)
                for P in order:
                    pool = s_psp2 if P % 3 == 2 else s_psp
                    ps = pool.tile([128, 1024], f32, tag="s")
                    e = e_p.tile([128, 1024], bf16, tag="e")
                    for h in range(2):
                        nc.tensor.matmul(
                            ps[:, 512 * h:512 * h + 512],
                            q8z4[:, 2 * P + h, :, :],
                            kq83[:, j, :][0:64, :]
                            .rearrange("p (i n) -> p i n", i=2),
                            start=True, stop=True,
                            perf_mode=DR, skip_group_check=True)
                    if (P * 3 + j) % 10 < SCHRAUD_TENTHS and P != j:
                        nc.vector.tensor_scalar(
                            out=e[:].bitcast(i16), in0=ps[:],
                            scalar1=SCH_C1, scalar2=SCH_C2,
                            op0=mybir.AluOpType.mult,
                            op1=mybir.AluOpType.add)
                    else:
                        nc.scalar.activation(e[:], ps[:], Exp, scale=S_SCALE)
                    if P == j:  # diagonal pair: mask cols [0:256], [768:1024]
                        src = e[:].rearrange("p (a n) -> p a n", n=256)[:, 0:4:3, :]
                        nc.gpsimd.tensor_mul(
                            src, src,
                            masks[:].rearrange("p (a n) -> p a n", n=256))
                    es[P] = e

            def pv_group(j, dj, tail=False):
                es = e_tiles[j]
                jd = 4 * j + dj
                if tail:  # S rings are idle after the last exp
                    pool = s_psp2 if dj % 3 == 2 else s_psp
                    po = pool.tile([128, 1024], f32, tag="s")
                else:
                    po = pv_psp.tile([128, 512], f32, tag="pv")
                for m in range(jd // 2 + 1):
                    e = es[m // 2]
                    col = 512 * (m % 2) + 128 * (jd % 4)
                    nc.tensor.matmul(
                        po[:, 0:65],
                        e[:, col:col + 128],
                        v3[:, m, :],
                        start=(m == 0), stop=(m == jd // 2),
                        skip_group_check=True)
                if j < 3:  # ACT idles in the ramp; DVE is the pacer later
                    nc.scalar.copy(out=outbuf[:, 65 * jd:65 * jd + 65],
                                   in_=po[:, 0:65])
                else:
                    nc.vector.tensor_copy(
                        out=outbuf[:, 65 * jd:65 * jd + 65],
                        in_=po[:, 0:65])

            def attn_pv(j, tail=False):
                for dj in range(4):
                    pv_group(j, dj, tail=tail)
                e_tiles.pop(j)

            out_r = out_d.rearrange("(jd t) h -> t jd h", t=128)
            def load_x(j):
                # x8 (kq passes 1-2 + v), then the compact own-cols residual
                # (v pass 3, two chunks per load), then full xl8 only for
                # late chunks' kq x-residual pass
                nc.sync.dma_start(xall3[:, 0:8, 512 * j:512 * (j + 1)],
                                  xalld3[:, 0:8, 512 * j:512 * (j + 1)])
                if j % 2 == 0:
                    g = slice(512 * (j // 2), 512 * (j // 2) + 512)
                    nc.sync.dma_start(xlo3[:, :, g], xlod3[:, :, g])
                if j >= JCUT:
                    nc.sync.dma_start(xall3[:, 8:16, 512 * j:512 * (j + 1)],
                                      xalld3[:, 8:16, 512 * j:512 * (j + 1)])

            def flush_out(j):
                # flush this chunk's 4 t-blocks
                nc.sync.dma_start(
                    out_r[:, 4 * j:4 * (j + 1), :],
                    outbuf[:, 260 * j:260 * (j + 1)]
                    .rearrange("p (jd h) -> p jd h", h=65))

            load_x(0)
            nc.sync.dma_start(bias[:], bias_d)
            nc.sync.dma_start(masks[:], mask_d)
            load_x(1)
            # software-pipelined: attn_s(j) issues right after proj(j) so
            # the first S/exp are never stuck behind a next-chunk proj that
            # waits on DMA; attn_pv trails by one chunk so the in-order PE
            # queue never waits on exp before the next chunk's proj/S.
            proj(0)
            for j in range(NCHUNK):
                attn_s(j)
                if j + 2 < NCHUNK:
                    load_x(j + 2)
                if j + 1 < NCHUNK:
                    proj(j + 1)
                if j > 0:
                    attn_pv(j - 1)
                    flush_out(j - 1)
            attn_pv(NCHUNK - 1, tail=True)
            flush_out(NCHUNK - 1)
    nc.compile()
    return nc


def _host_mask(p):
    """[128, 2, 256] bf16: [tri|ones] for p=0, [tri|zeros] for p=1."""
    s = np.arange(128)[:, None]
    c = np.arange(128)[None, :]
    tri = (s <= c).astype(np.float32)
    second = np.ones((128, 128), np.float32) if p == 0 else np.zeros((128, 128), np.float32)
    m = np.concatenate([tri, second], axis=1)  # [128, 256]
    return np.concatenate([m, m], axis=1).astype(BF16)  # [128, 512] = [2, 256]


def kernel(x, Wk, bk, Wq, bq, Wv, bv):
    from concourse.bass_utils import run_bass_kernel_spmd

    if "nc" not in _cache:
        _cache["nc"] = _build_program()
    nc = _cache["nc"]

    x = np.asarray(x, np.float32)
    wkq_f = A_W * np.concatenate([np.asarray(Wk), np.asarray(Wq)],
                                 axis=1).astype(np.float32)
    wkq = wkq_f.astype(F8)
    wkqb = (wkq_f / G_XL).astype(F8)
    wkql = (wkq_f - wkq.astype(np.float32)).astype(F8)
    wv_f = np.asarray(Wv, np.float32)
    wv8 = (A_W * wv_f).astype(F8)
    wv8b = (A_W / G_XL * wv_f).astype(F8)
    wvl = (A_W * wv_f - wv8.astype(np.float32)).astype(F8)
    shuf = lambda w: np.ascontiguousarray(
        w.reshape(8, 128, -1).transpose(1, 0, 2).reshape(128, -1))
    wpack = np.concatenate(
        [shuf(w) for w in (wkq, wkqb, wkql, wv8, wv8b, wvl)], axis=1)
    bias = np.zeros((128, 1), np.float32)
    bias[0:64, 0] = KQ_SCALE * np.asarray(bk, np.float32)

    in_maps = []
    for core in range(NCORES):
        b, p = core // 2, core % 2
        xb = x[b]  # [T, D]
        if p == 1:  # pair-swap 128-row blocks within 256-row pairs
            xb = xb.reshape(T // 256, 2, 128, D)[:, ::-1].reshape(T, D)
        xT = np.ascontiguousarray(xb.T)        # [D, T-device]
        x8 = xT.astype(F8)
        xl8 = (G_XL * (xT - x8.astype(np.float32))).astype(F8)
        xlo = np.ascontiguousarray(
            xl8.reshape(D, NCHUNK, 4, 128)[:, :, 0:3:2, :].reshape(D, T // 2))
        in_maps.append({
            "xall": np.ascontiguousarray(np.concatenate([x8, xl8], axis=0)),
            "xlo": xlo,
            "wpack": wpack,
            "bias": bias,
            "mask": _host_mask(p),
        })

    res = run_bass_kernel_spmd(nc, in_maps, core_ids=list(range(NCORES)))
    results = res.results
    _cache["last_run"] = res

    bv_f = np.asarray(bv, np.float32)
    out = np.zeros((B, T, H), np.float32)
    for b in range(B):
        a0 = results[2 * b]["out"].astype(np.float32)   # [T-device, 65]
        a1 = results[2 * b + 1]["out"].astype(np.float32)  # pair-swapped
        a1 = a1.reshape(T // 256, 2, 128, 65)[:, ::-1].reshape(T, 65)
        tot = a0 + a1
        # v was carried at scale A_W on-device
        out[b] = tot[:, 0:64] / (A_W * tot[:, 64:65]) + bv_f
    return out
